# revision 5
# baseline (speedup 1.0000x reference)
"""Trainium2 Bass kernel for nn_Decoding_Layer (dense transformer decoder layer).

Sharding: 8 cores = 4 batches x 2 sequence-halves. Each core computes one
512-row query block of one batch end-to-end (no collectives). K/V projections
are computed over the full 1024-key sequence per core; causal masking is
data-driven (host-fed mask slice for the diagonal 512x512 block plus a V-row
mask that zeroes invalid key blocks), so all 8 cores run one uniform program.

All big matmuls run as float32r (fp32 operands truncated to ~fp22 inside the
PE at full bf16-rate) with fp32 PSUM accumulation. Activations are kept
feature-major ("transposed", [feat, row]) so weights load untransposed and
per-feature biases fold into per-partition ACT bias slots during PSUM drains.
"""

import sys

if "/opt/trn_rl_repo" not in sys.path:
    sys.path.insert(0, "/opt/trn_rl_repo")

import numpy as np

import concourse.bass as bass
import concourse.mybir as mybir
import concourse.tile as tile
from concourse.bass_utils import run_bass_kernel_spmd

f32 = mybir.dt.float32
f32r = mybir.dt.float32r
AF = mybir.ActivationFunctionType
ALU = mybir.AluOpType

B, S, D, H, DFF = 4, 1024, 1024, 16, 4096
DEPTH = D // H
R = 512          # rows (query block) per core
EPS = 1e-6
N_CORES = 8


def _split_waits(nc, maxw=1):
    """Walrus in this toolchain encodes at most one semaphore wait per
    instruction; Tile emits several. Move excess waits onto same-engine NOPs
    placed immediately before the instruction (sequential per-engine streams
    make this equivalent)."""
    for f in nc.m.functions:
        for bb in f.blocks:
            out = []
            for inst in bb.instructions:
                si = inst.sync_info
                if si is not None and len(si.on_wait) > maxw:
                    waits = list(si.on_wait)
                    keep, excess = waits[-maxw:], waits[:-maxw]
                    eng = getattr(inst, "engine", None)
                    k = 0
                    while excess:
                        chunk, excess = excess[:maxw], excess[maxw:]
                        out.append(mybir.InstNoOp(
                            name=f"{inst.name}_wsp{k}",
                            engine=eng,
                            bass_nofuse=True,
                            sync_info=mybir.SyncInfo(on_wait=chunk, on_update=[]),
                        ))
                        k += 1
                    inst.sync_info = mybir.SyncInfo(
                        on_wait=keep, on_update=list(si.on_update))
                out.append(inst)
            bb.instructions = out


def build_program():
    nc = bass.Bass("TRN2", target_bir_lowering=False, debug=False)

    def din(name, shape):
        return nc.dram_tensor(name, shape, f32, kind="ExternalInput").ap()

    dc_own_d = din("dc_own", [D, R])        # dec_input own rows, transposed
    dke_d = din("dke", [D, S])              # dec keys (reordered: ctx|diag), transposed
    enc_d = din("encT", [D, S])             # enc_output, transposed
    mask_d = din("maskT", [R, R])           # causal diag block, [key, q], pre * -8e9
    padb_d = din("padb", [128, 8])          # -1e9 * padding_mask, chunked
    vm1_d = din("vm1", [128, 8])            # self V-row mask (chunked)
    vm1r_d = din("vm1r", [128, 8, 16])      # same, replicated per head
    vm2_d = din("vm2", [128, 8])            # ones
    vm2r_d = din("vm2r", [128, 8, 16])      # ones
    ones_d = din("onesd", [128, 128])       # ones
    w_d = {k: din(k, [D, D]) for k in ("wq1", "wk1", "wq2", "wk2", "wo1", "wo2")}
    fw1_d = din("fw1", [D, DFF])
    fw2_d = din("fw2", [DFF, D])
    bc_d = {k: din(k, [128, 8]) for k in
            ("bq1c", "bk1c", "bq2c", "bk2c", "bo1c", "bo2c", "fb2c",
             "g1c", "b1c", "g2c", "b2c", "g3c", "b3c")}
    fb1c_d = din("fb1c", [128, 32])
    out_d = nc.dram_tensor("outT", [D, R], f32, kind="ExternalOutput").ap()

    with tile.TileContext(nc) as tc:
        with tc.tile_pool(name="persist", bufs=1) as pp, \
             tc.tile_pool(name="consts", bufs=1) as cp:
            # ---- persistent SBUF ----
            arena = pp.tile([128, 16384], f32r, name="arena")     # 8 MiB
            dke = arena[:, 0:8192].rearrange("p (f r) -> p f r", f=8)
            kTv = arena[:, 8192:16384].rearrange("p (f r) -> p f r", f=8)
            vaug = pp.tile([128, 8, 16, 65], f32r, name="vaug")
            dc_own = pp.tile([128, 8, R], f32r, name="dc_own")
            qbuf = pp.tile([128, 8, R], f32r, name="qbuf")        # q1T -> q2T
            abufA = pp.tile([128, 8, R], f32r, name="abufA")      # attn1T -> x2pre/x2T
            xa = pp.tile([128, 8, R], f32r, name="xa")            # x1pre/x1T -> x3pre
            # phase-3/4 views of the arena: enc goes where k1T lived, k2T where
            # dec-keys lived, attn2T into the (then-dead) enc region.
            kT2v = arena[:, 0:8192].rearrange("p (f r) -> p f r", f=8)
            encv = arena[:, 8192:16384].rearrange("p (f r) -> p f r", f=8)
            abufB = arena[:, 8192:12288].rearrange("p (f r) -> p f r", f=8)

            # ---- constants ----
            onesb = cp.tile([128, 128], f32r, name="onesb")
            padb = cp.tile([128, 8], f32, name="padb")
            vm1 = cp.tile([128, 8], f32, name="vm1")
            vm2 = cp.tile([128, 8], f32, name="vm2")
            bcs = {k: cp.tile([128, 8], f32, name=k) for k in bc_d}
            fb1c = cp.tile([128, 32], f32, name="fb1c")

            nc.sync.dma_start(out=onesb, in_=ones_d.bitcast(f32r))
            nc.sync.dma_start(out=padb, in_=padb_d)
            nc.sync.dma_start(out=vm1, in_=vm1_d)
            nc.sync.dma_start(out=vm2, in_=vm2_d)
            for k in bcs:
                nc.sync.dma_start(out=bcs[k], in_=bc_d[k])
            nc.sync.dma_start(out=fb1c, in_=fb1c_d)
            nc.sync.dma_start(out=dc_own,
                              in_=dc_own_d.rearrange("(f p) r -> p f r", p=128).bitcast(f32r))
            nc.sync.dma_start(out=dke,
                              in_=dke_d.rearrange("(f p) r -> p f r", p=128).bitcast(f32r))

            ones1 = onesb[0:1, :]      # [1, 128] f32r
            onesp = onesb[:, 0:1]      # [128, 1] f32r

            # ---- helpers ----
            def gemm_TN(Wd, xt, KCn, MCn, NN, drain, wp, ps):
                """OUT^T[m-chunk, n] = sum_kc W[kc, m]^T @ xt(kc, n).
                xt(kc, n) -> [128, 512] f32r AP. drain(mi, n, psum_ap)."""
                g = max(1, 4 // NN)
                for mg in range(0, MCn, g):
                    gs = min(g, MCn - mg)
                    pps = {}
                    for i in range(gs):
                        for n in range(NN):
                            pps[(i, n)] = ps.tile([128, 512], f32, name="pp")
                    for kc in range(KCn):
                        wt = wp.tile([128, gs * 128], f32r, name="wt")
                        nc.sync.dma_start(
                            out=wt,
                            in_=Wd[kc * 128:(kc + 1) * 128,
                                   mg * 128:(mg + gs) * 128].bitcast(f32r))
                        for i in range(gs):
                            for n in range(NN):
                                nc.tensor.matmul(
                                    pps[(i, n)][:],
                                    lhsT=wt[:, i * 128:(i + 1) * 128],
                                    rhs=xt(kc, n),
                                    start=(kc == 0), stop=(kc == KCn - 1))
                    for i in range(gs):
                        for n in range(NN):
                            drain(mg + i, n, pps[(i, n)])

            def gemm_NT(Wd, xt_sb, KCn, RCn, NFn, drain, wp, ps):
                """OUT[r-chunk] = X @ W : lhsT = xt chunks, rhs = W cols.
                drain(rc, nf, psum_ap). xt_sb [128, KCn, S] f32r."""
                for nf in range(NFn):
                    for rg in range(0, RCn, 4):
                        gs = min(4, RCn - rg)
                        pps = [ps.tile([128, 512], f32, name="pp") for _ in range(gs)]
                        for kc in range(KCn):
                            wt = wp.tile([128, 512], f32r, name="wt")
                            nc.sync.dma_start(
                                out=wt,
                                in_=Wd[kc * 128:(kc + 1) * 128,
                                       nf * 512:(nf + 1) * 512].bitcast(f32r))
                            for i in range(gs):
                                nc.tensor.matmul(
                                    pps[i][:],
                                    lhsT=xt_sb[:, kc, (rg + i) * 128:(rg + i + 1) * 128],
                                    rhs=wt[:],
                                    start=(kc == 0), stop=(kc == KCn - 1))
                        for i in range(gs):
                            drain(rg + i, nf, pps[i])

            def attention(q_sb, kT_sb, v_sb, attn_out, is_self, masks, spool,
                          epool, upool, ps_s, ps_av, ps_b):
                for f in range(8):
                    avs = [ps_av.tile([65, 512], f32, name="av") for _ in range(2)]
                    for kc in range(8):
                        ss = [ps_s.tile([128, 512], f32, name="ss") for _ in range(2)]
                        for a in range(2):
                            nc.tensor.matmul(
                                ss[a][:],
                                lhsT=kT_sb[64 * a:64 * (a + 1), f, kc * 128:(kc + 1) * 128],
                                rhs=q_sb[64 * a:64 * (a + 1), f, :],
                                start=True, stop=True)
                        for a in range(2):
                            if is_self and kc >= 4:
                                sm = spool.tile([128, 512], f32, name="sm")
                                nc.vector.tensor_add(sm[:], ss[a][:], masks[:, kc - 4, :])
                                esrc = sm
                            else:
                                esrc = ss[a]
                            e = epool.tile([128, 512], f32r, name="ee")
                            bias = 0.0 if is_self else padb[:, kc:kc + 1]
                            nc.scalar.activation(e[:], esrc[:], AF.Exp,
                                                 bias=bias, scale=0.125)
                            nc.tensor.matmul(
                                avs[a][:],
                                lhsT=v_sb[:, kc, 2 * f + a, :],
                                rhs=e[:],
                                start=(kc == 0), stop=(kc == 7))
                    for a in range(2):
                        rec = upool.tile([1, 512], f32r, name="rec")
                        with nc.allow_low_precision(reason="f32r keeps fp32 bits"):
                            nc.vector.reciprocal(rec[:], avs[a][64:65, :])
                        dst = attn_out[64 * a:64 * (a + 1), f, :]
                        nc.scalar.copy(dst, avs[a][0:64, :])
                        bp = ps_b.tile([64, 512], f32, name="bp")
                        nc.tensor.matmul(bp[:], lhsT=onesb[0:1, 0:64], rhs=rec[:],
                                         start=True, stop=True)
                        with nc.allow_low_precision(reason="f32r keeps fp32 bits"):
                            nc.vector.tensor_mul(dst, dst, bp[:])

            def layernorm(x_sb, gC, bC, out_sb, sqp, ltp, ps_ln):
                pm = ps_ln.tile([1, 512], f32, name="pm")
                pv = ps_ln.tile([1, 512], f32, name="pv")
                for kc in range(8):
                    nc.tensor.matmul(pm[:], lhsT=onesp, rhs=x_sb[:, kc, :],
                                     start=(kc == 0), stop=(kc == 7))
                    sq = sqp.tile([128, 512], f32r, name="sq")
                    nc.scalar.activation(sq[:], x_sb[:, kc, :], AF.Square)
                    nc.tensor.matmul(pv[:], lhsT=onesp, rhs=sq[:],
                                     start=(kc == 0), stop=(kc == 7))
                m = ltp.tile([1, 512], f32, name="lm")
                sc = ltp.tile([1, 512], f32, name="lsc")
                sc2 = ltp.tile([1, 512], f32, name="lsc2")
                inv = ltp.tile([1, 512], f32r, name="linv")
                minv = ltp.tile([1, 512], f32r, name="lminv")
                nc.vector.tensor_scalar_mul(m[:], pm[:], 1.0 / D)
                nc.vector.tensor_scalar_mul(sc[:], pv[:], 1.0 / D)   # E[x^2]
                nc.vector.tensor_mul(sc2[:], m[:], m[:])             # m^2
                nc.vector.tensor_scalar_add(sc2[:], sc2[:], -EPS)
                nc.vector.tensor_tensor(out=sc[:], in0=sc[:], in1=sc2[:],
                                        op=ALU.subtract)             # var + eps
                nc.scalar.activation(sc[:], sc[:], AF.Sqrt)
                with nc.allow_low_precision(reason="f32r keeps fp32 bits"):
                    nc.vector.reciprocal(inv[:], sc[:])
                    nc.vector.tensor_mul(minv[:], m[:], inv[:])
                binv = ps_ln.tile([128, 512], f32, name="binv")
                bmv = ps_ln.tile([128, 512], f32, name="bmv")
                nc.tensor.matmul(binv[:], lhsT=ones1, rhs=inv[:], start=True, stop=True)
                nc.tensor.matmul(bmv[:], lhsT=ones1, rhs=minv[:], start=True, stop=True)
                for kc in range(8):
                    t1 = ltp.tile([128, 512], f32, name="lt1")
                    t2 = ltp.tile([128, 512], f32, name="lt2")
                    nc.vector.tensor_mul(t1[:], x_sb[:, kc, :], binv[:])
                    nc.vector.tensor_tensor(out=t2[:], in0=t1[:], in1=bmv[:],
                                            op=ALU.subtract)
                    nc.scalar.activation(out_sb[:, kc, :], t2[:], AF.Identity,
                                         bias=bC[:, kc:kc + 1], scale=gC[:, kc:kc + 1])

            # ================= phase 1: self projections =================
            with tc.tile_pool(name="w1", bufs=4) as wp, \
                 tc.tile_pool(name="ps1", bufs=6, space="PSUM") as ps:

                def drain_q1(mi, n, pa):
                    nc.scalar.activation(qbuf[:, mi, :], pa[:], AF.Identity,
                                         bias=bcs["bq1c"][:, mi:mi + 1])
                gemm_TN(w_d["wq1"], lambda kc, n: dc_own[:, kc, :], 8, 8, 1,
                        drain_q1, wp, ps)

                def drain_k1(mi, n, pa):
                    nc.scalar.activation(kTv[:, mi, n * 512:(n + 1) * 512], pa[:],
                                         AF.Identity, bias=bcs["bk1c"][:, mi:mi + 1])
                gemm_TN(w_d["wk1"], lambda kc, n: dke[:, kc, n * 512:(n + 1) * 512],
                        8, 8, 2, drain_k1, wp, ps)

                def drain_v1(rc, nf, pa):
                    dst = vaug[:, rc, nf * 8:(nf + 1) * 8, 0:64]
                    src = pa[:].rearrange("p (h d) -> p h d", h=8)
                    nc.scalar.activation(dst, src, AF.Copy, scale=vm1[:, rc:rc + 1])
                gemm_NT(w_d["wq1"], dke, 8, 8, 2, drain_v1, wp, ps)
                for rc in range(8):
                    nc.sync.dma_start(out=vaug[:, rc, :, 64:65],
                                      in_=vm1r_d[:, rc, :].bitcast(f32r))

            # ================= phase 2: self attention =================
            with tc.tile_pool(name="mk2", bufs=1) as mkp, \
                 tc.tile_pool(name="sp2", bufs=2) as spool, \
                 tc.tile_pool(name="ep2", bufs=3) as epool, \
                 tc.tile_pool(name="up2", bufs=2) as upool, \
                 tc.tile_pool(name="pss", bufs=4, space="PSUM") as ps_s, \
                 tc.tile_pool(name="psav", bufs=2, space="PSUM") as ps_av, \
                 tc.tile_pool(name="psb", bufs=2, space="PSUM") as ps_b:
                masks = mkp.tile([128, 4, R], f32, name="masks")
                for c in range(4):
                    nc.sync.dma_start(out=masks[:, c, :],
                                      in_=mask_d[c * 128:(c + 1) * 128, :])
                attention(qbuf, kTv, vaug, abufA, True, masks, spool, epool,
                          upool, ps_s, ps_av, ps_b)

            # ================= phase 3: cross projections =================
            with tc.tile_pool(name="w3", bufs=4) as wp, \
                 tc.tile_pool(name="ps3", bufs=6, space="PSUM") as ps:
                nc.sync.dma_start(out=encv,
                                  in_=enc_d.rearrange("(f p) r -> p f r", p=128).bitcast(f32r))

                def drain_q2(mi, n, pa):
                    nc.scalar.activation(qbuf[:, mi, :], pa[:], AF.Identity,
                                         bias=bcs["bq2c"][:, mi:mi + 1])
                gemm_TN(w_d["wq2"], lambda kc, n: dc_own[:, kc, :], 8, 8, 1,
                        drain_q2, wp, ps)

                def drain_k2(mi, n, pa):
                    nc.scalar.activation(kT2v[:, mi, n * 512:(n + 1) * 512], pa[:],
                                         AF.Identity, bias=bcs["bk2c"][:, mi:mi + 1])
                gemm_TN(w_d["wk2"], lambda kc, n: encv[:, kc, n * 512:(n + 1) * 512],
                        8, 8, 2, drain_k2, wp, ps)

                def drain_v2(rc, nf, pa):
                    dst = vaug[:, rc, nf * 8:(nf + 1) * 8, 0:64]
                    src = pa[:].rearrange("p (h d) -> p h d", h=8)
                    nc.scalar.activation(dst, src, AF.Copy, scale=vm2[:, rc:rc + 1])
                gemm_NT(w_d["wq2"], encv, 8, 8, 2, drain_v2, wp, ps)
                for rc in range(8):
                    nc.sync.dma_start(out=vaug[:, rc, :, 64:65],
                                      in_=vm2r_d[:, rc, :].bitcast(f32r))

            # ================= phase 4: cross attention =================
            with tc.tile_pool(name="sp4", bufs=2) as spool, \
                 tc.tile_pool(name="ep4", bufs=3) as epool, \
                 tc.tile_pool(name="up4", bufs=2) as upool, \
                 tc.tile_pool(name="pss4", bufs=4, space="PSUM") as ps_s, \
                 tc.tile_pool(name="psav4", bufs=2, space="PSUM") as ps_av, \
                 tc.tile_pool(name="psb4", bufs=2, space="PSUM") as ps_b:
                attention(qbuf, kT2v, vaug, abufB, False, None, spool, epool,
                          upool, ps_s, ps_av, ps_b)

            # ============ phase 5: output projections + LN1/LN2 ============
            with tc.tile_pool(name="w5", bufs=4) as wp, \
                 tc.tile_pool(name="tw5", bufs=2) as twp, \
                 tc.tile_pool(name="sq5", bufs=2) as sqp, \
                 tc.tile_pool(name="lt5", bufs=1) as ltp, \
                 tc.tile_pool(name="ps5", bufs=4, space="PSUM") as ps, \
                 tc.tile_pool(name="ps5ln", bufs=1, space="PSUM") as ps_ln:

                def drain_wo1(mi, n, pa):
                    tw = twp.tile([128, 512], f32, name="tw")
                    nc.scalar.activation(tw[:], pa[:], AF.Identity,
                                         bias=bcs["bo1c"][:, mi:mi + 1])
                    with nc.allow_low_precision(reason="f32r keeps fp32 bits"):
                        nc.vector.tensor_add(xa[:, mi, :], tw[:], dc_own[:, mi, :])
                gemm_TN(w_d["wo1"], lambda kc, n: abufA[:, kc, :], 8, 8, 1,
                        drain_wo1, wp, ps)

                layernorm(xa, bcs["g1c"], bcs["b1c"], xa, sqp, ltp, ps_ln)

                def drain_wo2(mi, n, pa):
                    tw = twp.tile([128, 512], f32, name="tw")
                    nc.scalar.activation(tw[:], pa[:], AF.Identity,
                                         bias=bcs["bo2c"][:, mi:mi + 1])
                    with nc.allow_low_precision(reason="f32r keeps fp32 bits"):
                        nc.vector.tensor_add(abufA[:, mi, :], tw[:], xa[:, mi, :])
                gemm_TN(w_d["wo2"], lambda kc, n: abufB[:, kc, :], 8, 8, 1,
                        drain_wo2, wp, ps)

                layernorm(abufA, bcs["g2c"], bcs["b2c"], abufA, sqp, ltp, ps_ln)

            # ================= phase 6: FFN + LN3 + output =================
            with tc.tile_pool(name="w6", bufs=4) as wp, \
                 tc.tile_pool(name="tw6", bufs=2) as twp, \
                 tc.tile_pool(name="sq6", bufs=2) as sqp, \
                 tc.tile_pool(name="lt6", bufs=1) as ltp, \
                 tc.tile_pool(name="ps6", bufs=6, space="PSUM") as ps:

                def drain_f1(mi, n, pa):
                    nc.scalar.activation(arena[:, mi * 512:(mi + 1) * 512], pa[:],
                                         AF.Relu, bias=fb1c[:, mi:mi + 1])
                gemm_TN(fw1_d, lambda kc, n: abufA[:, kc, :], 8, 32, 1,
                        drain_f1, wp, ps)

            with tc.tile_pool(name="w6b", bufs=4) as wp, \
                 tc.tile_pool(name="tw6b", bufs=2) as twp, \
                 tc.tile_pool(name="sq6b", bufs=2) as sqp, \
                 tc.tile_pool(name="lt6b", bufs=1) as ltp, \
                 tc.tile_pool(name="ps6b", bufs=1, space="PSUM") as ps8:
                pps = [ps8.tile([128, 512], f32, name=f"pf{i}") for i in range(8)]
                for kc in range(32):
                    for mh in range(2):
                        wt = wp.tile([128, 512], f32r, name="wt")
                        nc.sync.dma_start(
                            out=wt,
                            in_=fw2_d[kc * 128:(kc + 1) * 128,
                                      mh * 512:(mh + 1) * 512].bitcast(f32r))
                        for i in range(4):
                            nc.tensor.matmul(
                                pps[mh * 4 + i][:],
                                lhsT=wt[:, i * 128:(i + 1) * 128],
                                rhs=arena[:, kc * 512:(kc + 1) * 512],
                                start=(kc == 0), stop=(kc == 31))
                for mi in range(8):
                    tw = twp.tile([128, 512], f32, name="tw")
                    nc.scalar.activation(tw[:], pps[mi][:], AF.Identity,
                                         bias=bcs["fb2c"][:, mi:mi + 1])
                    with nc.allow_low_precision(reason="f32r keeps fp32 bits"):
                        nc.vector.tensor_add(xa[:, mi, :], tw[:], abufA[:, mi, :])

            with tc.tile_pool(name="sq7", bufs=2) as sqp, \
                 tc.tile_pool(name="lt7", bufs=1) as ltp, \
                 tc.tile_pool(name="ps7ln", bufs=1, space="PSUM") as ps_ln:
                layernorm(xa, bcs["g3c"], bcs["b3c"], qbuf, sqp, ltp, ps_ln)
                for mi in range(8):
                    nc.sync.dma_start(out=out_d[mi * 128:(mi + 1) * 128, :].bitcast(f32r),
                                      in_=qbuf[:, mi, :])

    _split_waits(nc, 1)
    return nc


_PROGRAM = None


def _get_program():
    global _PROGRAM
    if _PROGRAM is None:
        _PROGRAM = build_program()
    return _PROGRAM


def _core_inputs(inp, c):
    b, j = c // 2, c % 2
    dec = np.asarray(inp["dec_input"][b], np.float32)      # [S, D]
    enc = np.asarray(inp["enc_output"][b], np.float32)
    decT = np.ascontiguousarray(dec.T)                     # [D, S]
    own = np.ascontiguousarray(decT[:, j * R:(j + 1) * R])
    if j == 1:
        dke = decT                                         # ctx = rows 0:512, diag = 512:1024
    else:
        dke = np.ascontiguousarray(
            np.concatenate([decT[:, R:], decT[:, :R]], axis=1))
    la = np.asarray(inp["look_ahead_mask"], np.float32)[0, 0]
    maskT = np.ascontiguousarray(la[j * R:(j + 1) * R, j * R:(j + 1) * R].T) * np.float32(-8e9)
    padb = (np.asarray(inp["padding_mask"], np.float32)[b, 0, 0] * np.float32(-1e9))
    vm = np.ones(S, np.float32)
    if j == 0:
        vm[:R] = 0.0                                       # ctx block invalid for first half
    v2 = np.ones(S, np.float32)

    def chunk(a, n):
        return np.ascontiguousarray(np.asarray(a, np.float32).reshape(n, 128).T)

    wo1 = np.asarray(inp["wo1"], np.float32)
    wo2 = np.asarray(inp["wo2"], np.float32)
    bo1e = np.asarray(inp["bq1"], np.float32) @ wo1 + np.asarray(inp["bo1"], np.float32)
    bo2e = np.asarray(inp["bq2"], np.float32) @ wo2 + np.asarray(inp["bo2"], np.float32)

    return {
        "dc_own": own, "dke": dke,
        "encT": np.ascontiguousarray(enc.T),
        "maskT": maskT,
        "padb": chunk(padb, 8),
        "vm1": chunk(vm, 8),
        "vm1r": np.repeat(chunk(vm, 8)[:, :, None], 16, axis=2),
        "vm2": chunk(v2, 8),
        "vm2r": np.ones((128, 8, 16), np.float32),
        "onesd": np.ones((128, 128), np.float32),
        "wq1": np.asarray(inp["wq1"], np.float32),
        "wk1": np.asarray(inp["wk1"], np.float32),
        "wq2": np.asarray(inp["wq2"], np.float32),
        "wk2": np.asarray(inp["wk2"], np.float32),
        "wo1": wo1, "wo2": wo2,
        "fw1": np.asarray(inp["ff_w1"], np.float32),
        "fw2": np.asarray(inp["ff_w2"], np.float32),
        "bq1c": chunk(inp["bq1"], 8), "bk1c": chunk(inp["bk1"], 8),
        "bq2c": chunk(inp["bq2"], 8), "bk2c": chunk(inp["bk2"], 8),
        "bo1c": chunk(bo1e, 8), "bo2c": chunk(bo2e, 8),
        "fb1c": chunk(inp["ff_b1"], 32), "fb2c": chunk(inp["ff_b2"], 8),
        "g1c": chunk(inp["ln1_g"], 8), "b1c": chunk(inp["ln1_b"], 8),
        "g2c": chunk(inp["ln2_g"], 8), "b2c": chunk(inp["ln2_b"], 8),
        "g3c": chunk(inp["ln3_g"], 8), "b3c": chunk(inp["ln3_b"], 8),
    }


def kernel(**inputs):
    nc = _get_program()
    in_maps = [_core_inputs(inputs, c) for c in range(N_CORES)]
    res = run_bass_kernel_spmd(nc, in_maps, list(range(N_CORES)))
    out = np.empty((B, S, D), np.float32)
    for c in range(N_CORES):
        b, j = c // 2, c % 2
        out[b, j * R:(j + 1) * R, :] = res.results[c]["outT"].T
    return out


if __name__ == "__main__":
    import tempfile
    from concourse.bass_utils import compile_bass_kernel
    nc = build_program()
    with tempfile.TemporaryDirectory() as td:
        compile_bass_kernel(nc, td)
    print("COMPILE OK")


# revision 6
# speedup vs baseline: 1.0058x; 1.0058x over previous
"""Trainium2 Bass kernel for nn_Decoding_Layer (dense transformer decoder layer).

Sharding: 8 cores = 4 batches x 2 sequence-halves. Each core computes one
512-row query block of one batch end-to-end (no collectives). K/V projections
are computed over the full 1024-key sequence per core; causal masking is
data-driven (host-fed mask slice for the diagonal 512x512 block plus a V-row
mask that zeroes invalid key blocks), so all 8 cores run one uniform program.

All big matmuls run as float32r (fp32 operands truncated to ~fp22 inside the
PE at full bf16-rate) with fp32 PSUM accumulation. Activations are kept
feature-major ("transposed", [feat, row]) so weights load untransposed and
per-feature biases fold into per-partition ACT bias slots during PSUM drains.
"""

import sys

if "/opt/trn_rl_repo" not in sys.path:
    sys.path.insert(0, "/opt/trn_rl_repo")

import numpy as np

import concourse.bass as bass
import concourse.mybir as mybir
import concourse.tile as tile
from concourse import bass_utils
from concourse.bass_utils import run_bass_kernel_spmd

# walrus ships with --enable-ldw-opt=false; enabling it lets codegen overlap
# the per-matmul 4-byte weight loads, which otherwise serialize with the
# matmul stream on this fp32r-heavy kernel.
_orig_run_command = bass_utils.run_command

def _patched_run_command(argv, **kw):
    argv = ["--enable-ldw-opt=true" if a == "--enable-ldw-opt=false" else a
            for a in argv]
    return _orig_run_command(argv, **kw)

bass_utils.run_command = _patched_run_command

f32 = mybir.dt.float32
f32r = mybir.dt.float32r
AF = mybir.ActivationFunctionType
ALU = mybir.AluOpType

B, S, D, H, DFF = 4, 1024, 1024, 16, 4096
DEPTH = D // H
R = 512          # rows (query block) per core
EPS = 1e-6
N_CORES = 8


def _split_waits(nc, maxw=1):
    """Walrus in this toolchain encodes at most one semaphore wait per
    instruction; Tile emits several. Move excess waits onto same-engine NOPs
    placed immediately before the instruction (sequential per-engine streams
    make this equivalent)."""
    for f in nc.m.functions:
        for bb in f.blocks:
            out = []
            for inst in bb.instructions:
                si = inst.sync_info
                if si is not None and len(si.on_wait) > maxw:
                    waits = list(si.on_wait)
                    keep, excess = waits[-maxw:], waits[:-maxw]
                    eng = getattr(inst, "engine", None)
                    k = 0
                    while excess:
                        chunk, excess = excess[:maxw], excess[maxw:]
                        out.append(mybir.InstNoOp(
                            name=f"{inst.name}_wsp{k}",
                            engine=eng,
                            bass_nofuse=True,
                            sync_info=mybir.SyncInfo(on_wait=chunk, on_update=[]),
                        ))
                        k += 1
                    inst.sync_info = mybir.SyncInfo(
                        on_wait=keep, on_update=list(si.on_update))
                out.append(inst)
            bb.instructions = out


def build_program():
    nc = bass.Bass("TRN2", target_bir_lowering=False, debug=False)

    def din(name, shape):
        return nc.dram_tensor(name, shape, f32, kind="ExternalInput").ap()

    dc_own_d = din("dc_own", [D, R])        # dec_input own rows, transposed
    dke_d = din("dke", [D, S])              # dec keys (reordered: ctx|diag), transposed
    enc_d = din("encT", [D, S])             # enc_output, transposed
    mask_d = din("maskT", [R, R])           # causal diag block, [key, q], pre * -8e9
    padb_d = din("padb", [128, 8])          # -1e9 * padding_mask, chunked
    vm1_d = din("vm1", [128, 8])            # self V-row mask (chunked)
    vm1r_d = din("vm1r", [128, 8, 16])      # same, replicated per head
    vm2_d = din("vm2", [128, 8])            # ones
    vm2r_d = din("vm2r", [128, 8, 16])      # ones
    ones_d = din("onesd", [128, 128])       # ones
    w_d = {k: din(k, [D, D]) for k in ("wq1", "wk1", "wq2", "wk2", "wo1", "wo2")}
    fw1_d = din("fw1", [D, DFF])
    fw2_d = din("fw2", [DFF, D])
    bc_d = {k: din(k, [128, 8]) for k in
            ("bq1c", "bk1c", "bq2c", "bk2c", "bo1c", "bo2c", "fb2c",
             "g1c", "b1c", "g2c", "b2c", "g3c", "b3c")}
    fb1c_d = din("fb1c", [128, 32])
    out_d = nc.dram_tensor("outT", [D, R], f32, kind="ExternalOutput").ap()

    with tile.TileContext(nc) as tc:
        with tc.tile_pool(name="persist", bufs=1) as pp, \
             tc.tile_pool(name="consts", bufs=1) as cp:
            # ---- persistent SBUF ----
            arena = pp.tile([128, 16384], f32r, name="arena")     # 8 MiB
            dke = arena[:, 0:8192].rearrange("p (f r) -> p f r", f=8)
            kTv = arena[:, 8192:16384].rearrange("p (f r) -> p f r", f=8)
            vaug = pp.tile([128, 8, 16, 65], f32r, name="vaug")
            dc_own = pp.tile([128, 8, R], f32r, name="dc_own")
            qbuf = pp.tile([128, 8, R], f32r, name="qbuf")        # q1T -> q2T
            abufA = pp.tile([128, 8, R], f32r, name="abufA")      # attn1T -> x2pre/x2T
            xa = pp.tile([128, 8, R], f32r, name="xa")            # x1pre/x1T -> x3pre
            # phase-3/4 views of the arena: enc goes where k1T lived, k2T where
            # dec-keys lived, attn2T into the (then-dead) enc region.
            kT2v = arena[:, 0:8192].rearrange("p (f r) -> p f r", f=8)
            encv = arena[:, 8192:16384].rearrange("p (f r) -> p f r", f=8)
            abufB = arena[:, 8192:12288].rearrange("p (f r) -> p f r", f=8)

            # ---- constants ----
            onesb = cp.tile([128, 128], f32r, name="onesb")
            padb = cp.tile([128, 8], f32, name="padb")
            vm1 = cp.tile([128, 8], f32, name="vm1")
            vm2 = cp.tile([128, 8], f32, name="vm2")
            bcs = {k: cp.tile([128, 8], f32, name=k) for k in bc_d}
            fb1c = cp.tile([128, 32], f32, name="fb1c")

            nc.sync.dma_start(out=onesb, in_=ones_d.bitcast(f32r))
            nc.sync.dma_start(out=padb, in_=padb_d)
            nc.sync.dma_start(out=vm1, in_=vm1_d)
            nc.sync.dma_start(out=vm2, in_=vm2_d)
            for k in bcs:
                nc.sync.dma_start(out=bcs[k], in_=bc_d[k])
            nc.sync.dma_start(out=fb1c, in_=fb1c_d)
            nc.sync.dma_start(out=dc_own,
                              in_=dc_own_d.rearrange("(f p) r -> p f r", p=128).bitcast(f32r))
            nc.sync.dma_start(out=dke,
                              in_=dke_d.rearrange("(f p) r -> p f r", p=128).bitcast(f32r))

            ones1 = onesb[0:1, :]      # [1, 128] f32r
            onesp = onesb[:, 0:1]      # [128, 1] f32r

            # ---- helpers ----
            def gemm_TN(Wd, xt, KCn, MCn, NN, drain, wp, ps):
                """OUT^T[m-chunk, n] = sum_kc W[kc, m]^T @ xt(kc, n).
                xt(kc, n) -> [128, 512] f32r AP. drain(mi, n, psum_ap)."""
                g = max(1, 4 // NN)
                for mg in range(0, MCn, g):
                    gs = min(g, MCn - mg)
                    pps = {}
                    for i in range(gs):
                        for n in range(NN):
                            pps[(i, n)] = ps.tile([128, 512], f32, name="pp")
                    for kc in range(KCn):
                        wt = wp.tile([128, gs * 128], f32r, name="wt")
                        nc.sync.dma_start(
                            out=wt,
                            in_=Wd[kc * 128:(kc + 1) * 128,
                                   mg * 128:(mg + gs) * 128].bitcast(f32r))
                        for i in range(gs):
                            for n in range(NN):
                                nc.tensor.matmul(
                                    pps[(i, n)][:],
                                    lhsT=wt[:, i * 128:(i + 1) * 128],
                                    rhs=xt(kc, n),
                                    start=(kc == 0), stop=(kc == KCn - 1))
                    for i in range(gs):
                        for n in range(NN):
                            drain(mg + i, n, pps[(i, n)])

            def gemm_NT(Wd, xt_sb, KCn, RCn, NFn, drain, wp, ps):
                """OUT[r-chunk] = X @ W : lhsT = xt chunks, rhs = W cols.
                drain(rc, nf, psum_ap). xt_sb [128, KCn, S] f32r."""
                for nf in range(NFn):
                    for rg in range(0, RCn, 4):
                        gs = min(4, RCn - rg)
                        pps = [ps.tile([128, 512], f32, name="pp") for _ in range(gs)]
                        for kc in range(KCn):
                            wt = wp.tile([128, 512], f32r, name="wt")
                            nc.sync.dma_start(
                                out=wt,
                                in_=Wd[kc * 128:(kc + 1) * 128,
                                       nf * 512:(nf + 1) * 512].bitcast(f32r))
                            for i in range(gs):
                                nc.tensor.matmul(
                                    pps[i][:],
                                    lhsT=xt_sb[:, kc, (rg + i) * 128:(rg + i + 1) * 128],
                                    rhs=wt[:],
                                    start=(kc == 0), stop=(kc == KCn - 1))
                        for i in range(gs):
                            drain(rg + i, nf, pps[i])

            def attention(q_sb, kT_sb, v_sb, attn_out, is_self, masks, spool,
                          epool, upool, ps_s, ps_av, ps_b):
                for f in range(8):
                    avs = [ps_av.tile([65, 512], f32, name="av") for _ in range(2)]
                    for kc in range(8):
                        ss = [ps_s.tile([128, 512], f32, name="ss") for _ in range(2)]
                        for a in range(2):
                            nc.tensor.matmul(
                                ss[a][:],
                                lhsT=kT_sb[64 * a:64 * (a + 1), f, kc * 128:(kc + 1) * 128],
                                rhs=q_sb[64 * a:64 * (a + 1), f, :],
                                start=True, stop=True)
                        for a in range(2):
                            if is_self and kc >= 4:
                                sm = spool.tile([128, 512], f32, name="sm")
                                nc.vector.tensor_add(sm[:], ss[a][:], masks[:, kc - 4, :])
                                esrc = sm
                            else:
                                esrc = ss[a]
                            e = epool.tile([128, 512], f32r, name="ee")
                            bias = 0.0 if is_self else padb[:, kc:kc + 1]
                            nc.scalar.activation(e[:], esrc[:], AF.Exp,
                                                 bias=bias, scale=0.125)
                            nc.tensor.matmul(
                                avs[a][:],
                                lhsT=v_sb[:, kc, 2 * f + a, :],
                                rhs=e[:],
                                start=(kc == 0), stop=(kc == 7))
                    for a in range(2):
                        rec = upool.tile([1, 512], f32r, name="rec")
                        with nc.allow_low_precision(reason="f32r keeps fp32 bits"):
                            nc.vector.reciprocal(rec[:], avs[a][64:65, :])
                        dst = attn_out[64 * a:64 * (a + 1), f, :]
                        nc.scalar.copy(dst, avs[a][0:64, :])
                        bp = ps_b.tile([64, 512], f32, name="bp")
                        nc.tensor.matmul(bp[:], lhsT=onesb[0:1, 0:64], rhs=rec[:],
                                         start=True, stop=True)
                        with nc.allow_low_precision(reason="f32r keeps fp32 bits"):
                            nc.vector.tensor_mul(dst, dst, bp[:])

            def layernorm(x_sb, gC, bC, out_sb, sqp, ltp, ps_ln):
                pm = ps_ln.tile([1, 512], f32, name="pm")
                pv = ps_ln.tile([1, 512], f32, name="pv")
                for kc in range(8):
                    nc.tensor.matmul(pm[:], lhsT=onesp, rhs=x_sb[:, kc, :],
                                     start=(kc == 0), stop=(kc == 7))
                    sq = sqp.tile([128, 512], f32r, name="sq")
                    nc.scalar.activation(sq[:], x_sb[:, kc, :], AF.Square)
                    nc.tensor.matmul(pv[:], lhsT=onesp, rhs=sq[:],
                                     start=(kc == 0), stop=(kc == 7))
                m = ltp.tile([1, 512], f32, name="lm")
                sc = ltp.tile([1, 512], f32, name="lsc")
                sc2 = ltp.tile([1, 512], f32, name="lsc2")
                inv = ltp.tile([1, 512], f32r, name="linv")
                minv = ltp.tile([1, 512], f32r, name="lminv")
                nc.vector.tensor_scalar_mul(m[:], pm[:], 1.0 / D)
                nc.vector.tensor_scalar_mul(sc[:], pv[:], 1.0 / D)   # E[x^2]
                nc.vector.tensor_mul(sc2[:], m[:], m[:])             # m^2
                nc.vector.tensor_scalar_add(sc2[:], sc2[:], -EPS)
                nc.vector.tensor_tensor(out=sc[:], in0=sc[:], in1=sc2[:],
                                        op=ALU.subtract)             # var + eps
                nc.scalar.activation(sc[:], sc[:], AF.Sqrt)
                with nc.allow_low_precision(reason="f32r keeps fp32 bits"):
                    nc.vector.reciprocal(inv[:], sc[:])
                    nc.vector.tensor_mul(minv[:], m[:], inv[:])
                binv = ps_ln.tile([128, 512], f32, name="binv")
                bmv = ps_ln.tile([128, 512], f32, name="bmv")
                nc.tensor.matmul(binv[:], lhsT=ones1, rhs=inv[:], start=True, stop=True)
                nc.tensor.matmul(bmv[:], lhsT=ones1, rhs=minv[:], start=True, stop=True)
                for kc in range(8):
                    t1 = ltp.tile([128, 512], f32, name="lt1")
                    t2 = ltp.tile([128, 512], f32, name="lt2")
                    nc.vector.tensor_mul(t1[:], x_sb[:, kc, :], binv[:])
                    nc.vector.tensor_tensor(out=t2[:], in0=t1[:], in1=bmv[:],
                                            op=ALU.subtract)
                    nc.scalar.activation(out_sb[:, kc, :], t2[:], AF.Identity,
                                         bias=bC[:, kc:kc + 1], scale=gC[:, kc:kc + 1])

            # ================= phase 1: self projections =================
            with tc.tile_pool(name="w1", bufs=4) as wp, \
                 tc.tile_pool(name="ps1", bufs=6, space="PSUM") as ps:

                def drain_q1(mi, n, pa):
                    nc.scalar.activation(qbuf[:, mi, :], pa[:], AF.Identity,
                                         bias=bcs["bq1c"][:, mi:mi + 1])
                gemm_TN(w_d["wq1"], lambda kc, n: dc_own[:, kc, :], 8, 8, 1,
                        drain_q1, wp, ps)

                def drain_k1(mi, n, pa):
                    nc.scalar.activation(kTv[:, mi, n * 512:(n + 1) * 512], pa[:],
                                         AF.Identity, bias=bcs["bk1c"][:, mi:mi + 1])
                gemm_TN(w_d["wk1"], lambda kc, n: dke[:, kc, n * 512:(n + 1) * 512],
                        8, 8, 2, drain_k1, wp, ps)

                def drain_v1(rc, nf, pa):
                    dst = vaug[:, rc, nf * 8:(nf + 1) * 8, 0:64]
                    src = pa[:].rearrange("p (h d) -> p h d", h=8)
                    nc.scalar.activation(dst, src, AF.Copy, scale=vm1[:, rc:rc + 1])
                gemm_NT(w_d["wq1"], dke, 8, 8, 2, drain_v1, wp, ps)
                for rc in range(8):
                    nc.sync.dma_start(out=vaug[:, rc, :, 64:65],
                                      in_=vm1r_d[:, rc, :].bitcast(f32r))

            # ================= phase 2: self attention =================
            with tc.tile_pool(name="mk2", bufs=1) as mkp, \
                 tc.tile_pool(name="sp2", bufs=2) as spool, \
                 tc.tile_pool(name="ep2", bufs=3) as epool, \
                 tc.tile_pool(name="up2", bufs=2) as upool, \
                 tc.tile_pool(name="pss", bufs=4, space="PSUM") as ps_s, \
                 tc.tile_pool(name="psav", bufs=2, space="PSUM") as ps_av, \
                 tc.tile_pool(name="psb", bufs=2, space="PSUM") as ps_b:
                masks = mkp.tile([128, 4, R], f32, name="masks")
                for c in range(4):
                    nc.sync.dma_start(out=masks[:, c, :],
                                      in_=mask_d[c * 128:(c + 1) * 128, :])
                attention(qbuf, kTv, vaug, abufA, True, masks, spool, epool,
                          upool, ps_s, ps_av, ps_b)

            # ================= phase 3: cross projections =================
            with tc.tile_pool(name="w3", bufs=4) as wp, \
                 tc.tile_pool(name="ps3", bufs=6, space="PSUM") as ps:
                nc.sync.dma_start(out=encv,
                                  in_=enc_d.rearrange("(f p) r -> p f r", p=128).bitcast(f32r))

                def drain_q2(mi, n, pa):
                    nc.scalar.activation(qbuf[:, mi, :], pa[:], AF.Identity,
                                         bias=bcs["bq2c"][:, mi:mi + 1])
                gemm_TN(w_d["wq2"], lambda kc, n: dc_own[:, kc, :], 8, 8, 1,
                        drain_q2, wp, ps)

                def drain_k2(mi, n, pa):
                    nc.scalar.activation(kT2v[:, mi, n * 512:(n + 1) * 512], pa[:],
                                         AF.Identity, bias=bcs["bk2c"][:, mi:mi + 1])
                gemm_TN(w_d["wk2"], lambda kc, n: encv[:, kc, n * 512:(n + 1) * 512],
                        8, 8, 2, drain_k2, wp, ps)

                def drain_v2(rc, nf, pa):
                    dst = vaug[:, rc, nf * 8:(nf + 1) * 8, 0:64]
                    src = pa[:].rearrange("p (h d) -> p h d", h=8)
                    nc.scalar.activation(dst, src, AF.Copy, scale=vm2[:, rc:rc + 1])
                gemm_NT(w_d["wq2"], encv, 8, 8, 2, drain_v2, wp, ps)
                for rc in range(8):
                    nc.sync.dma_start(out=vaug[:, rc, :, 64:65],
                                      in_=vm2r_d[:, rc, :].bitcast(f32r))

            # ================= phase 4: cross attention =================
            with tc.tile_pool(name="sp4", bufs=2) as spool, \
                 tc.tile_pool(name="ep4", bufs=3) as epool, \
                 tc.tile_pool(name="up4", bufs=2) as upool, \
                 tc.tile_pool(name="pss4", bufs=4, space="PSUM") as ps_s, \
                 tc.tile_pool(name="psav4", bufs=2, space="PSUM") as ps_av, \
                 tc.tile_pool(name="psb4", bufs=2, space="PSUM") as ps_b:
                attention(qbuf, kT2v, vaug, abufB, False, None, spool, epool,
                          upool, ps_s, ps_av, ps_b)

            # ============ phase 5: output projections + LN1/LN2 ============
            with tc.tile_pool(name="w5", bufs=4) as wp, \
                 tc.tile_pool(name="tw5", bufs=2) as twp, \
                 tc.tile_pool(name="sq5", bufs=2) as sqp, \
                 tc.tile_pool(name="lt5", bufs=1) as ltp, \
                 tc.tile_pool(name="ps5", bufs=4, space="PSUM") as ps, \
                 tc.tile_pool(name="ps5ln", bufs=1, space="PSUM") as ps_ln:

                def drain_wo1(mi, n, pa):
                    tw = twp.tile([128, 512], f32, name="tw")
                    nc.scalar.activation(tw[:], pa[:], AF.Identity,
                                         bias=bcs["bo1c"][:, mi:mi + 1])
                    with nc.allow_low_precision(reason="f32r keeps fp32 bits"):
                        nc.vector.tensor_add(xa[:, mi, :], tw[:], dc_own[:, mi, :])
                gemm_TN(w_d["wo1"], lambda kc, n: abufA[:, kc, :], 8, 8, 1,
                        drain_wo1, wp, ps)

                layernorm(xa, bcs["g1c"], bcs["b1c"], xa, sqp, ltp, ps_ln)

                def drain_wo2(mi, n, pa):
                    tw = twp.tile([128, 512], f32, name="tw")
                    nc.scalar.activation(tw[:], pa[:], AF.Identity,
                                         bias=bcs["bo2c"][:, mi:mi + 1])
                    with nc.allow_low_precision(reason="f32r keeps fp32 bits"):
                        nc.vector.tensor_add(abufA[:, mi, :], tw[:], xa[:, mi, :])
                gemm_TN(w_d["wo2"], lambda kc, n: abufB[:, kc, :], 8, 8, 1,
                        drain_wo2, wp, ps)

                layernorm(abufA, bcs["g2c"], bcs["b2c"], abufA, sqp, ltp, ps_ln)

            # ================= phase 6: FFN + LN3 + output =================
            with tc.tile_pool(name="w6", bufs=4) as wp, \
                 tc.tile_pool(name="tw6", bufs=2) as twp, \
                 tc.tile_pool(name="sq6", bufs=2) as sqp, \
                 tc.tile_pool(name="lt6", bufs=1) as ltp, \
                 tc.tile_pool(name="ps6", bufs=6, space="PSUM") as ps:

                def drain_f1(mi, n, pa):
                    nc.scalar.activation(arena[:, mi * 512:(mi + 1) * 512], pa[:],
                                         AF.Relu, bias=fb1c[:, mi:mi + 1])
                gemm_TN(fw1_d, lambda kc, n: abufA[:, kc, :], 8, 32, 1,
                        drain_f1, wp, ps)

            with tc.tile_pool(name="w6b", bufs=4) as wp, \
                 tc.tile_pool(name="tw6b", bufs=2) as twp, \
                 tc.tile_pool(name="sq6b", bufs=2) as sqp, \
                 tc.tile_pool(name="lt6b", bufs=1) as ltp, \
                 tc.tile_pool(name="ps6b", bufs=1, space="PSUM") as ps8:
                pps = [ps8.tile([128, 512], f32, name=f"pf{i}") for i in range(8)]
                for kc in range(32):
                    for mh in range(2):
                        wt = wp.tile([128, 512], f32r, name="wt")
                        nc.sync.dma_start(
                            out=wt,
                            in_=fw2_d[kc * 128:(kc + 1) * 128,
                                      mh * 512:(mh + 1) * 512].bitcast(f32r))
                        for i in range(4):
                            nc.tensor.matmul(
                                pps[mh * 4 + i][:],
                                lhsT=wt[:, i * 128:(i + 1) * 128],
                                rhs=arena[:, kc * 512:(kc + 1) * 512],
                                start=(kc == 0), stop=(kc == 31))
                for mi in range(8):
                    tw = twp.tile([128, 512], f32, name="tw")
                    nc.scalar.activation(tw[:], pps[mi][:], AF.Identity,
                                         bias=bcs["fb2c"][:, mi:mi + 1])
                    with nc.allow_low_precision(reason="f32r keeps fp32 bits"):
                        nc.vector.tensor_add(xa[:, mi, :], tw[:], abufA[:, mi, :])

            with tc.tile_pool(name="sq7", bufs=2) as sqp, \
                 tc.tile_pool(name="lt7", bufs=1) as ltp, \
                 tc.tile_pool(name="ps7ln", bufs=1, space="PSUM") as ps_ln:
                layernorm(xa, bcs["g3c"], bcs["b3c"], qbuf, sqp, ltp, ps_ln)
                for mi in range(8):
                    nc.sync.dma_start(out=out_d[mi * 128:(mi + 1) * 128, :].bitcast(f32r),
                                      in_=qbuf[:, mi, :])

    _split_waits(nc, 1)
    return nc


_PROGRAM = None


def _get_program():
    global _PROGRAM
    if _PROGRAM is None:
        _PROGRAM = build_program()
    return _PROGRAM


def _core_inputs(inp, c):
    b, j = c // 2, c % 2
    dec = np.asarray(inp["dec_input"][b], np.float32)      # [S, D]
    enc = np.asarray(inp["enc_output"][b], np.float32)
    decT = np.ascontiguousarray(dec.T)                     # [D, S]
    own = np.ascontiguousarray(decT[:, j * R:(j + 1) * R])
    if j == 1:
        dke = decT                                         # ctx = rows 0:512, diag = 512:1024
    else:
        dke = np.ascontiguousarray(
            np.concatenate([decT[:, R:], decT[:, :R]], axis=1))
    la = np.asarray(inp["look_ahead_mask"], np.float32)[0, 0]
    maskT = np.ascontiguousarray(la[j * R:(j + 1) * R, j * R:(j + 1) * R].T) * np.float32(-8e9)
    padb = (np.asarray(inp["padding_mask"], np.float32)[b, 0, 0] * np.float32(-1e9))
    vm = np.ones(S, np.float32)
    if j == 0:
        vm[:R] = 0.0                                       # ctx block invalid for first half
    v2 = np.ones(S, np.float32)

    def chunk(a, n):
        return np.ascontiguousarray(np.asarray(a, np.float32).reshape(n, 128).T)

    wo1 = np.asarray(inp["wo1"], np.float32)
    wo2 = np.asarray(inp["wo2"], np.float32)
    bo1e = np.asarray(inp["bq1"], np.float32) @ wo1 + np.asarray(inp["bo1"], np.float32)
    bo2e = np.asarray(inp["bq2"], np.float32) @ wo2 + np.asarray(inp["bo2"], np.float32)

    return {
        "dc_own": own, "dke": dke,
        "encT": np.ascontiguousarray(enc.T),
        "maskT": maskT,
        "padb": chunk(padb, 8),
        "vm1": chunk(vm, 8),
        "vm1r": np.repeat(chunk(vm, 8)[:, :, None], 16, axis=2),
        "vm2": chunk(v2, 8),
        "vm2r": np.ones((128, 8, 16), np.float32),
        "onesd": np.ones((128, 128), np.float32),
        "wq1": np.asarray(inp["wq1"], np.float32),
        "wk1": np.asarray(inp["wk1"], np.float32),
        "wq2": np.asarray(inp["wq2"], np.float32),
        "wk2": np.asarray(inp["wk2"], np.float32),
        "wo1": wo1, "wo2": wo2,
        "fw1": np.asarray(inp["ff_w1"], np.float32),
        "fw2": np.asarray(inp["ff_w2"], np.float32),
        "bq1c": chunk(inp["bq1"], 8), "bk1c": chunk(inp["bk1"], 8),
        "bq2c": chunk(inp["bq2"], 8), "bk2c": chunk(inp["bk2"], 8),
        "bo1c": chunk(bo1e, 8), "bo2c": chunk(bo2e, 8),
        "fb1c": chunk(inp["ff_b1"], 32), "fb2c": chunk(inp["ff_b2"], 8),
        "g1c": chunk(inp["ln1_g"], 8), "b1c": chunk(inp["ln1_b"], 8),
        "g2c": chunk(inp["ln2_g"], 8), "b2c": chunk(inp["ln2_b"], 8),
        "g3c": chunk(inp["ln3_g"], 8), "b3c": chunk(inp["ln3_b"], 8),
    }


def kernel(**inputs):
    nc = _get_program()
    in_maps = [_core_inputs(inputs, c) for c in range(N_CORES)]
    res = run_bass_kernel_spmd(nc, in_maps, list(range(N_CORES)))
    out = np.empty((B, S, D), np.float32)
    for c in range(N_CORES):
        b, j = c // 2, c % 2
        out[b, j * R:(j + 1) * R, :] = res.results[c]["outT"].T
    return out


if __name__ == "__main__":
    import tempfile
    from concourse.bass_utils import compile_bass_kernel
    nc = build_program()
    with tempfile.TemporaryDirectory() as td:
        compile_bass_kernel(nc, td)
    print("COMPILE OK")


# revision 7
# speedup vs baseline: 1.1483x; 1.1416x over previous
"""Trainium2 Bass kernel for nn_Decoding_Layer (dense transformer decoder layer).

Sharding: 8 cores = 4 batches x 2 sequence-halves. Each core computes one
512-row query block of one batch end-to-end (no collectives). K/V projections
are computed over the full 1024-key sequence per core; causal masking is
data-driven (host-fed mask slice for the diagonal 512x512 block plus a V-row
mask that zeroes invalid key blocks), so all 8 cores run one uniform program.

All big matmuls run as float32r (fp32 operands truncated to ~fp22 inside the
PE at full bf16-rate) with fp32 PSUM accumulation. Activations are kept
feature-major ("transposed", [feat, row]) so weights load untransposed and
per-feature biases fold into per-partition ACT bias slots during PSUM drains.
"""

import sys

if "/opt/trn_rl_repo" not in sys.path:
    sys.path.insert(0, "/opt/trn_rl_repo")

import numpy as np

import concourse.bass as bass
import concourse.mybir as mybir
import concourse.tile as tile
from concourse import bass_utils
from concourse.bass_utils import run_bass_kernel_spmd

# walrus ships with --enable-ldw-opt=false; enabling it lets codegen overlap
# the per-matmul 4-byte weight loads, which otherwise serialize with the
# matmul stream on this fp32r-heavy kernel.
_orig_run_command = bass_utils.run_command

def _patched_run_command(argv, **kw):
    argv = ["--enable-ldw-opt=true" if a == "--enable-ldw-opt=false" else a
            for a in argv]
    return _orig_run_command(argv, **kw)

bass_utils.run_command = _patched_run_command

f32 = mybir.dt.float32
f32r = mybir.dt.float32r
AF = mybir.ActivationFunctionType
ALU = mybir.AluOpType

B, S, D, H, DFF = 4, 1024, 1024, 16, 4096
DEPTH = D // H
R = 512          # rows (query block) per core
EPS = 1e-6
N_CORES = 8


def _split_waits(nc, maxw=1):
    """Walrus in this toolchain encodes at most one semaphore wait per
    instruction; Tile emits several. Move excess waits onto same-engine NOPs
    placed immediately before the instruction (sequential per-engine streams
    make this equivalent)."""
    for f in nc.m.functions:
        for bb in f.blocks:
            out = []
            for inst in bb.instructions:
                si = inst.sync_info
                if si is not None and len(si.on_wait) > maxw:
                    waits = list(si.on_wait)
                    keep, excess = waits[-maxw:], waits[:-maxw]
                    eng = getattr(inst, "engine", None)
                    k = 0
                    while excess:
                        chunk, excess = excess[:maxw], excess[maxw:]
                        out.append(mybir.InstNoOp(
                            name=f"{inst.name}_wsp{k}",
                            engine=eng,
                            bass_nofuse=True,
                            sync_info=mybir.SyncInfo(on_wait=chunk, on_update=[]),
                        ))
                        k += 1
                    inst.sync_info = mybir.SyncInfo(
                        on_wait=keep, on_update=list(si.on_update))
                out.append(inst)
            bb.instructions = out


def build_program():
    nc = bass.Bass("TRN2", target_bir_lowering=False, debug=False)

    def din(name, shape):
        return nc.dram_tensor(name, shape, f32, kind="ExternalInput").ap()

    dc_own_d = din("dc_own", [D, R])        # dec_input own rows, transposed
    dke_d = din("dke", [D, S])              # dec keys (reordered: ctx|diag), transposed
    enc_d = din("encT", [D, S])             # enc_output, transposed
    mask_d = din("maskT", [R, R])           # causal diag block, [key, q], pre * -8e9
    padb_d = din("padb", [128, 8])          # -1e9 * padding_mask, chunked
    vm1_d = din("vm1", [128, 8])            # self V-row mask (chunked)
    vm1r_d = din("vm1r", [128, 8, 16])      # same, replicated per head
    vm2_d = din("vm2", [128, 8])            # ones
    vm2r_d = din("vm2r", [128, 8, 16])      # ones
    ones_d = din("onesd", [128, 128])       # ones
    w_d = {k: din(k, [D, D]) for k in ("wq1", "wk1", "wq2", "wk2", "wo1", "wo2")}
    fw1_d = din("fw1", [D, DFF])
    fw2_d = din("fw2", [DFF, D])
    bc_d = {k: din(k, [128, 8]) for k in
            ("bq1c", "bk1c", "bq2c", "bk2c", "bo1c", "bo2c", "fb2c",
             "g1c", "b1c", "g2c", "b2c", "g3c", "b3c")}
    fb1c_d = din("fb1c", [128, 32])
    out_d = nc.dram_tensor("outT", [D, R], f32, kind="ExternalOutput").ap()

    with tile.TileContext(nc) as tc:
        with tc.tile_pool(name="persist", bufs=1) as pp, \
             tc.tile_pool(name="consts", bufs=1) as cp:
            # ---- persistent SBUF ----
            arena = pp.tile([128, 16384], f32r, name="arena")     # 8 MiB
            dke = arena[:, 0:8192].rearrange("p (f r) -> p f r", f=8)
            kTv = arena[:, 8192:16384].rearrange("p (f r) -> p f r", f=8)
            vaug = pp.tile([128, 8, 16, 65], f32r, name="vaug")
            dc_own = pp.tile([128, 8, R], f32r, name="dc_own")
            qbuf = pp.tile([128, 8, R], f32r, name="qbuf")        # q1T -> q2T
            abufA = pp.tile([128, 8, R], f32r, name="abufA")      # attn1T -> x2pre/x2T
            xa = pp.tile([128, 8, R], f32r, name="xa")            # x1pre/x1T -> x3pre
            # phase-3/4 views of the arena: enc goes where k1T lived, k2T where
            # dec-keys lived, attn2T into the (then-dead) enc region.
            kT2v = arena[:, 0:8192].rearrange("p (f r) -> p f r", f=8)
            encv = arena[:, 8192:16384].rearrange("p (f r) -> p f r", f=8)
            abufB = arena[:, 8192:12288].rearrange("p (f r) -> p f r", f=8)

            # ---- constants ----
            onesb = cp.tile([128, 128], f32r, name="onesb")
            padb = cp.tile([128, 8], f32, name="padb")
            vm1 = cp.tile([128, 8], f32, name="vm1")
            vm2 = cp.tile([128, 8], f32, name="vm2")
            bcs = {k: cp.tile([128, 8], f32, name=k) for k in bc_d}
            fb1c = cp.tile([128, 32], f32, name="fb1c")

            nc.sync.dma_start(out=onesb, in_=ones_d.bitcast(f32r))
            nc.sync.dma_start(out=padb, in_=padb_d)
            nc.sync.dma_start(out=vm1, in_=vm1_d)
            nc.sync.dma_start(out=vm2, in_=vm2_d)
            for k in bcs:
                nc.sync.dma_start(out=bcs[k], in_=bc_d[k])
            nc.sync.dma_start(out=fb1c, in_=fb1c_d)
            nc.sync.dma_start(out=dc_own,
                              in_=dc_own_d.rearrange("(f p) r -> p f r", p=128).bitcast(f32r))
            nc.sync.dma_start(out=dke,
                              in_=dke_d.rearrange("(f p) r -> p f r", p=128).bitcast(f32r))

            ones1 = onesb[0:1, :]      # [1, 128] f32r
            onesp = onesb[:, 0:1]      # [128, 1] f32r

            # ---- helpers ----
            def gemm_TN(Wd, xt, KCn, MCn, NN, drain, wp, ps):
                """OUT^T[m-chunk, n] = sum_kc W[kc, m]^T @ xt(kc, n).
                xt(kc, n) -> [128, 512] f32r AP. drain(mi, n, psum_ap)."""
                g = max(1, 4 // NN)
                for mg in range(0, MCn, g):
                    gs = min(g, MCn - mg)
                    pps = {}
                    for i in range(gs):
                        for n in range(NN):
                            pps[(i, n)] = ps.tile([128, 512], f32, name="pp")
                    for kc in range(KCn):
                        wt = wp.tile([128, gs * 128], f32r, name="wt")
                        nc.sync.dma_start(
                            out=wt,
                            in_=Wd[kc * 128:(kc + 1) * 128,
                                   mg * 128:(mg + gs) * 128].bitcast(f32r))
                        for i in range(gs):
                            for n in range(NN):
                                nc.tensor.matmul(
                                    pps[(i, n)][:],
                                    lhsT=wt[:, i * 128:(i + 1) * 128],
                                    rhs=xt(kc, n),
                                    start=(kc == 0), stop=(kc == KCn - 1))
                    for i in range(gs):
                        for n in range(NN):
                            drain(mg + i, n, pps[(i, n)])

            def gemm_NT(Wd, xt_sb, KCn, RCn, NFn, drain, wp, ps):
                """OUT[r-chunk] = X @ W : lhsT = xt chunks, rhs = W cols.
                drain(rc, nf, psum_ap). xt_sb [128, KCn, S] f32r."""
                for nf in range(NFn):
                    for rg in range(0, RCn, 4):
                        gs = min(4, RCn - rg)
                        pps = [ps.tile([128, 512], f32, name="pp") for _ in range(gs)]
                        for kc in range(KCn):
                            wt = wp.tile([128, 512], f32r, name="wt")
                            nc.sync.dma_start(
                                out=wt,
                                in_=Wd[kc * 128:(kc + 1) * 128,
                                       nf * 512:(nf + 1) * 512].bitcast(f32r))
                            for i in range(gs):
                                nc.tensor.matmul(
                                    pps[i][:],
                                    lhsT=xt_sb[:, kc, (rg + i) * 128:(rg + i + 1) * 128],
                                    rhs=wt[:],
                                    start=(kc == 0), stop=(kc == KCn - 1))
                        for i in range(gs):
                            drain(rg + i, nf, pps[i])

            def attention(q_sb, kT_sb, v_sb, attn_out, is_self, masks, spool,
                          epool, upool, ps_s, ps_av, ps_b):
                for f in range(8):
                    avs = [ps_av.tile([65, 512], f32, name="av") for _ in range(2)]
                    for kc in range(8):
                        ss = ps_s.tile([128, 1024], f32, name="ss")
                        for a in range(2):
                            nc.tensor.matmul(
                                ss[:, a * 512:(a + 1) * 512],
                                lhsT=kT_sb[64 * a:64 * (a + 1), f, kc * 128:(kc + 1) * 128],
                                rhs=q_sb[64 * a:64 * (a + 1), f, :],
                                start=True, stop=True)
                        if is_self and kc >= 4:
                            nc.vector.tensor_tensor(
                                out=ss[:].rearrange("p (a r) -> p a r", a=2),
                                in0=ss[:].rearrange("p (a r) -> p a r", a=2),
                                in1=bass.AP(tensor=masks.tensor,
                                            offset=masks[:, kc - 4, :].offset,
                                            ap=[list(masks.ap[0]), [0, 2],
                                                list(masks.ap[2])]),
                                op=ALU.add)
                        e = epool.tile([128, 1024], f32r, name="ee")
                        bias = 0.0 if is_self else padb[:, kc:kc + 1]
                        nc.scalar.activation(e[:], ss[:], AF.Exp,
                                             bias=bias, scale=0.125)
                        for a in range(2):
                            nc.tensor.matmul(
                                avs[a][:],
                                lhsT=v_sb[:, kc, 2 * f + a, :],
                                rhs=e[:, a * 512:(a + 1) * 512],
                                start=(kc == 0), stop=(kc == 7))
                    for a in range(2):
                        rec = upool.tile([1, 512], f32r, name="rec")
                        with nc.allow_low_precision(reason="f32r keeps fp32 bits"):
                            nc.vector.reciprocal(rec[:], avs[a][64:65, :])
                        dst = attn_out[64 * a:64 * (a + 1), f, :]
                        nc.scalar.copy(dst, avs[a][0:64, :])
                        bp = ps_b.tile([64, 512], f32, name="bp")
                        nc.tensor.matmul(bp[:], lhsT=onesb[0:1, 0:64], rhs=rec[:],
                                         start=True, stop=True)
                        with nc.allow_low_precision(reason="f32r keeps fp32 bits"):
                            nc.vector.tensor_mul(dst, dst, bp[:])

            def layernorm(x_sb, gC, bC, out_sb, sqp, ltp, ps_ln):
                pm = ps_ln.tile([1, 512], f32, name="pm")
                pv = ps_ln.tile([1, 512], f32, name="pv")
                for kc in range(8):
                    nc.tensor.matmul(pm[:], lhsT=onesp, rhs=x_sb[:, kc, :],
                                     start=(kc == 0), stop=(kc == 7))
                    sq = sqp.tile([128, 512], f32r, name="sq")
                    nc.scalar.activation(sq[:], x_sb[:, kc, :], AF.Square)
                    nc.tensor.matmul(pv[:], lhsT=onesp, rhs=sq[:],
                                     start=(kc == 0), stop=(kc == 7))
                m = ltp.tile([1, 512], f32, name="lm")
                sc = ltp.tile([1, 512], f32, name="lsc")
                sc2 = ltp.tile([1, 512], f32, name="lsc2")
                inv = ltp.tile([1, 512], f32r, name="linv")
                minv = ltp.tile([1, 512], f32r, name="lminv")
                nc.vector.tensor_scalar_mul(m[:], pm[:], 1.0 / D)
                nc.vector.tensor_scalar_mul(sc[:], pv[:], 1.0 / D)   # E[x^2]
                nc.vector.tensor_mul(sc2[:], m[:], m[:])             # m^2
                nc.vector.tensor_scalar_add(sc2[:], sc2[:], -EPS)
                nc.vector.tensor_tensor(out=sc[:], in0=sc[:], in1=sc2[:],
                                        op=ALU.subtract)             # var + eps
                nc.scalar.activation(sc[:], sc[:], AF.Sqrt)
                with nc.allow_low_precision(reason="f32r keeps fp32 bits"):
                    nc.vector.reciprocal(inv[:], sc[:])
                    nc.vector.tensor_mul(minv[:], m[:], inv[:])
                binv = ps_ln.tile([128, 512], f32, name="binv")
                bmv = ps_ln.tile([128, 512], f32, name="bmv")
                nc.tensor.matmul(binv[:], lhsT=ones1, rhs=inv[:], start=True, stop=True)
                nc.tensor.matmul(bmv[:], lhsT=ones1, rhs=minv[:], start=True, stop=True)
                def bc8(ps_t):
                    return bass.AP(tensor=ps_t.tensor, offset=ps_t.offset,
                                   ap=[list(ps_t.ap[0]), [0, 8], list(ps_t.ap[1])])
                with nc.allow_low_precision(reason="f32r keeps fp32 bits"):
                    nc.vector.tensor_tensor(out=x_sb[:], in0=x_sb[:], in1=bc8(binv),
                                            op=ALU.mult)
                    nc.vector.tensor_tensor(out=x_sb[:], in0=x_sb[:], in1=bc8(bmv),
                                            op=ALU.subtract)
                for kc in range(8):
                    nc.scalar.activation(out_sb[:, kc, :], x_sb[:, kc, :], AF.Identity,
                                         bias=bC[:, kc:kc + 1], scale=gC[:, kc:kc + 1])

            # ================= phase 1: self projections =================
            with tc.tile_pool(name="w1", bufs=6) as wp, \
                 tc.tile_pool(name="ps1", bufs=6, space="PSUM") as ps:

                def drain_q1(mi, n, pa):
                    nc.scalar.activation(qbuf[:, mi, :], pa[:], AF.Identity,
                                         bias=bcs["bq1c"][:, mi:mi + 1])
                gemm_TN(w_d["wq1"], lambda kc, n: dc_own[:, kc, :], 8, 8, 1,
                        drain_q1, wp, ps)

                def drain_k1(mi, n, pa):
                    nc.scalar.activation(kTv[:, mi, n * 512:(n + 1) * 512], pa[:],
                                         AF.Identity, bias=bcs["bk1c"][:, mi:mi + 1])
                gemm_TN(w_d["wk1"], lambda kc, n: dke[:, kc, n * 512:(n + 1) * 512],
                        8, 8, 2, drain_k1, wp, ps)

                def drain_v1(rc, nf, pa):
                    dst = vaug[:, rc, nf * 8:(nf + 1) * 8, 0:64]
                    src = pa[:].rearrange("p (h d) -> p h d", h=8)
                    nc.scalar.activation(dst, src, AF.Copy, scale=vm1[:, rc:rc + 1])
                gemm_NT(w_d["wq1"], dke, 8, 8, 2, drain_v1, wp, ps)
                for rc in range(8):
                    nc.sync.dma_start(out=vaug[:, rc, :, 64:65],
                                      in_=vm1r_d[:, rc, :].bitcast(f32r))

            # ================= phase 2: self attention =================
            with tc.tile_pool(name="mk2", bufs=1) as mkp, \
                 tc.tile_pool(name="sp2", bufs=2) as spool, \
                 tc.tile_pool(name="ep2", bufs=3) as epool, \
                 tc.tile_pool(name="up2", bufs=2) as upool, \
                 tc.tile_pool(name="pss", bufs=2, space="PSUM") as ps_s, \
                 tc.tile_pool(name="psav", bufs=2, space="PSUM") as ps_av, \
                 tc.tile_pool(name="psb", bufs=2, space="PSUM") as ps_b:
                masks = mkp.tile([128, 4, R], f32, name="masks")
                for c in range(4):
                    nc.sync.dma_start(out=masks[:, c, :],
                                      in_=mask_d[c * 128:(c + 1) * 128, :])
                attention(qbuf, kTv, vaug, abufA, True, masks, spool, epool,
                          upool, ps_s, ps_av, ps_b)

            # ================= phase 3: cross projections =================
            with tc.tile_pool(name="w3", bufs=6) as wp, \
                 tc.tile_pool(name="ps3", bufs=6, space="PSUM") as ps:
                nc.sync.dma_start(out=encv,
                                  in_=enc_d.rearrange("(f p) r -> p f r", p=128).bitcast(f32r))

                def drain_q2(mi, n, pa):
                    nc.scalar.activation(qbuf[:, mi, :], pa[:], AF.Identity,
                                         bias=bcs["bq2c"][:, mi:mi + 1])
                gemm_TN(w_d["wq2"], lambda kc, n: dc_own[:, kc, :], 8, 8, 1,
                        drain_q2, wp, ps)

                def drain_k2(mi, n, pa):
                    nc.scalar.activation(kT2v[:, mi, n * 512:(n + 1) * 512], pa[:],
                                         AF.Identity, bias=bcs["bk2c"][:, mi:mi + 1])
                gemm_TN(w_d["wk2"], lambda kc, n: encv[:, kc, n * 512:(n + 1) * 512],
                        8, 8, 2, drain_k2, wp, ps)

                def drain_v2(rc, nf, pa):
                    dst = vaug[:, rc, nf * 8:(nf + 1) * 8, 0:64]
                    src = pa[:].rearrange("p (h d) -> p h d", h=8)
                    nc.scalar.activation(dst, src, AF.Copy, scale=vm2[:, rc:rc + 1])
                gemm_NT(w_d["wq2"], encv, 8, 8, 2, drain_v2, wp, ps)
                for rc in range(8):
                    nc.sync.dma_start(out=vaug[:, rc, :, 64:65],
                                      in_=vm2r_d[:, rc, :].bitcast(f32r))

            # ================= phase 4: cross attention =================
            with tc.tile_pool(name="sp4", bufs=2) as spool, \
                 tc.tile_pool(name="ep4", bufs=3) as epool, \
                 tc.tile_pool(name="up4", bufs=2) as upool, \
                 tc.tile_pool(name="pss4", bufs=2, space="PSUM") as ps_s, \
                 tc.tile_pool(name="psav4", bufs=2, space="PSUM") as ps_av, \
                 tc.tile_pool(name="psb4", bufs=2, space="PSUM") as ps_b:
                attention(qbuf, kT2v, vaug, abufB, False, None, spool, epool,
                          upool, ps_s, ps_av, ps_b)

            # ============ phase 5: output projections + LN1/LN2 ============
            with tc.tile_pool(name="w5", bufs=6) as wp, \
                 tc.tile_pool(name="tw5", bufs=2) as twp, \
                 tc.tile_pool(name="sq5", bufs=2) as sqp, \
                 tc.tile_pool(name="lt5", bufs=1) as ltp, \
                 tc.tile_pool(name="ps5", bufs=4, space="PSUM") as ps, \
                 tc.tile_pool(name="ps5ln", bufs=1, space="PSUM") as ps_ln:

                def drain_wo1(mi, n, pa):
                    tw = twp.tile([128, 512], f32, name="tw")
                    nc.scalar.activation(tw[:], pa[:], AF.Identity,
                                         bias=bcs["bo1c"][:, mi:mi + 1])
                    with nc.allow_low_precision(reason="f32r keeps fp32 bits"):
                        nc.vector.tensor_add(xa[:, mi, :], tw[:], dc_own[:, mi, :])
                gemm_TN(w_d["wo1"], lambda kc, n: abufA[:, kc, :], 8, 8, 1,
                        drain_wo1, wp, ps)

                layernorm(xa, bcs["g1c"], bcs["b1c"], xa, sqp, ltp, ps_ln)

                def drain_wo2(mi, n, pa):
                    tw = twp.tile([128, 512], f32, name="tw")
                    nc.scalar.activation(tw[:], pa[:], AF.Identity,
                                         bias=bcs["bo2c"][:, mi:mi + 1])
                    with nc.allow_low_precision(reason="f32r keeps fp32 bits"):
                        nc.vector.tensor_add(abufA[:, mi, :], tw[:], xa[:, mi, :])
                gemm_TN(w_d["wo2"], lambda kc, n: abufB[:, kc, :], 8, 8, 1,
                        drain_wo2, wp, ps)

                layernorm(abufA, bcs["g2c"], bcs["b2c"], abufA, sqp, ltp, ps_ln)

            # ================= phase 6: FFN + LN3 + output =================
            with tc.tile_pool(name="w6", bufs=6) as wp, \
                 tc.tile_pool(name="tw6", bufs=2) as twp, \
                 tc.tile_pool(name="sq6", bufs=2) as sqp, \
                 tc.tile_pool(name="lt6", bufs=1) as ltp, \
                 tc.tile_pool(name="ps6", bufs=6, space="PSUM") as ps:

                def drain_f1(mi, n, pa):
                    nc.scalar.activation(arena[:, mi * 512:(mi + 1) * 512], pa[:],
                                         AF.Relu, bias=fb1c[:, mi:mi + 1])
                gemm_TN(fw1_d, lambda kc, n: abufA[:, kc, :], 8, 32, 1,
                        drain_f1, wp, ps)

            with tc.tile_pool(name="w6b", bufs=6) as wp, \
                 tc.tile_pool(name="tw6b", bufs=2) as twp, \
                 tc.tile_pool(name="sq6b", bufs=2) as sqp, \
                 tc.tile_pool(name="lt6b", bufs=1) as ltp, \
                 tc.tile_pool(name="ps6b", bufs=1, space="PSUM") as ps8:
                pps = [ps8.tile([128, 512], f32, name=f"pf{i}") for i in range(8)]
                for kc in range(32):
                    for mh in range(2):
                        wt = wp.tile([128, 512], f32r, name="wt")
                        nc.sync.dma_start(
                            out=wt,
                            in_=fw2_d[kc * 128:(kc + 1) * 128,
                                      mh * 512:(mh + 1) * 512].bitcast(f32r))
                        for i in range(4):
                            nc.tensor.matmul(
                                pps[mh * 4 + i][:],
                                lhsT=wt[:, i * 128:(i + 1) * 128],
                                rhs=arena[:, kc * 512:(kc + 1) * 512],
                                start=(kc == 0), stop=(kc == 31))
                for mi in range(8):
                    tw = twp.tile([128, 512], f32, name="tw")
                    nc.scalar.activation(tw[:], pps[mi][:], AF.Identity,
                                         bias=bcs["fb2c"][:, mi:mi + 1])
                    with nc.allow_low_precision(reason="f32r keeps fp32 bits"):
                        nc.vector.tensor_add(xa[:, mi, :], tw[:], abufA[:, mi, :])

            with tc.tile_pool(name="sq7", bufs=2) as sqp, \
                 tc.tile_pool(name="lt7", bufs=1) as ltp, \
                 tc.tile_pool(name="ps7ln", bufs=1, space="PSUM") as ps_ln:
                layernorm(xa, bcs["g3c"], bcs["b3c"], qbuf, sqp, ltp, ps_ln)
                for mi in range(8):
                    nc.sync.dma_start(out=out_d[mi * 128:(mi + 1) * 128, :].bitcast(f32r),
                                      in_=qbuf[:, mi, :])

    _split_waits(nc, 1)
    return nc


_PROGRAM = None


def _get_program():
    global _PROGRAM
    if _PROGRAM is None:
        _PROGRAM = build_program()
    return _PROGRAM


def _core_inputs(inp, c):
    b, j = c // 2, c % 2
    dec = np.asarray(inp["dec_input"][b], np.float32)      # [S, D]
    enc = np.asarray(inp["enc_output"][b], np.float32)
    decT = np.ascontiguousarray(dec.T)                     # [D, S]
    own = np.ascontiguousarray(decT[:, j * R:(j + 1) * R])
    if j == 1:
        dke = decT                                         # ctx = rows 0:512, diag = 512:1024
    else:
        dke = np.ascontiguousarray(
            np.concatenate([decT[:, R:], decT[:, :R]], axis=1))
    la = np.asarray(inp["look_ahead_mask"], np.float32)[0, 0]
    maskT = np.ascontiguousarray(la[j * R:(j + 1) * R, j * R:(j + 1) * R].T) * np.float32(-8e9)
    padb = (np.asarray(inp["padding_mask"], np.float32)[b, 0, 0] * np.float32(-1e9))
    vm = np.ones(S, np.float32)
    if j == 0:
        vm[:R] = 0.0                                       # ctx block invalid for first half
    v2 = np.ones(S, np.float32)

    def chunk(a, n):
        return np.ascontiguousarray(np.asarray(a, np.float32).reshape(n, 128).T)

    wo1 = np.asarray(inp["wo1"], np.float32)
    wo2 = np.asarray(inp["wo2"], np.float32)
    bo1e = np.asarray(inp["bq1"], np.float32) @ wo1 + np.asarray(inp["bo1"], np.float32)
    bo2e = np.asarray(inp["bq2"], np.float32) @ wo2 + np.asarray(inp["bo2"], np.float32)

    return {
        "dc_own": own, "dke": dke,
        "encT": np.ascontiguousarray(enc.T),
        "maskT": maskT,
        "padb": chunk(padb, 8),
        "vm1": chunk(vm, 8),
        "vm1r": np.repeat(chunk(vm, 8)[:, :, None], 16, axis=2),
        "vm2": chunk(v2, 8),
        "vm2r": np.ones((128, 8, 16), np.float32),
        "onesd": np.ones((128, 128), np.float32),
        "wq1": np.asarray(inp["wq1"], np.float32),
        "wk1": np.asarray(inp["wk1"], np.float32),
        "wq2": np.asarray(inp["wq2"], np.float32),
        "wk2": np.asarray(inp["wk2"], np.float32),
        "wo1": wo1, "wo2": wo2,
        "fw1": np.asarray(inp["ff_w1"], np.float32),
        "fw2": np.asarray(inp["ff_w2"], np.float32),
        "bq1c": chunk(inp["bq1"], 8), "bk1c": chunk(inp["bk1"], 8),
        "bq2c": chunk(inp["bq2"], 8), "bk2c": chunk(inp["bk2"], 8),
        "bo1c": chunk(bo1e, 8), "bo2c": chunk(bo2e, 8),
        "fb1c": chunk(inp["ff_b1"], 32), "fb2c": chunk(inp["ff_b2"], 8),
        "g1c": chunk(inp["ln1_g"], 8), "b1c": chunk(inp["ln1_b"], 8),
        "g2c": chunk(inp["ln2_g"], 8), "b2c": chunk(inp["ln2_b"], 8),
        "g3c": chunk(inp["ln3_g"], 8), "b3c": chunk(inp["ln3_b"], 8),
    }


def kernel(**inputs):
    nc = _get_program()
    in_maps = [_core_inputs(inputs, c) for c in range(N_CORES)]
    res = run_bass_kernel_spmd(nc, in_maps, list(range(N_CORES)))
    out = np.empty((B, S, D), np.float32)
    for c in range(N_CORES):
        b, j = c // 2, c % 2
        out[b, j * R:(j + 1) * R, :] = res.results[c]["outT"].T
    return out


if __name__ == "__main__":
    import tempfile
    from concourse.bass_utils import compile_bass_kernel
    nc = build_program()
    with tempfile.TemporaryDirectory() as td:
        compile_bass_kernel(nc, td)
    print("COMPILE OK")


# revision 9
# speedup vs baseline: 1.1889x; 1.0354x over previous
"""Trainium2 Bass kernel for nn_Decoding_Layer (dense transformer decoder layer).

Sharding: 8 cores = 4 batches x 2 sequence-halves. Each core computes one
512-row query block of one batch end-to-end (no collectives). K/V projections
are computed over the full 1024-key sequence per core; causal masking is
data-driven (host-fed mask slice for the diagonal 512x512 block plus a V-row
mask that zeroes invalid key blocks), so all 8 cores run one uniform program.

All big matmuls run as float32r (fp32 operands truncated to ~fp22 inside the
PE at full bf16-rate) with fp32 PSUM accumulation. Activations are kept
feature-major ("transposed", [feat, row]) so weights load untransposed and
per-feature biases fold into per-partition ACT bias slots during PSUM drains.
"""

import sys

if "/opt/trn_rl_repo" not in sys.path:
    sys.path.insert(0, "/opt/trn_rl_repo")

import numpy as np

import concourse.bass as bass
import concourse.mybir as mybir
import concourse.tile as tile
from concourse import bass_utils
from concourse.bass_utils import run_bass_kernel_spmd

# walrus ships with --enable-ldw-opt=false; enabling it lets codegen overlap
# the per-matmul 4-byte weight loads, which otherwise serialize with the
# matmul stream on this fp32r-heavy kernel.
_orig_run_command = bass_utils.run_command

def _patched_run_command(argv, **kw):
    argv = ["--enable-ldw-opt=true" if a == "--enable-ldw-opt=false" else a
            for a in argv]
    return _orig_run_command(argv, **kw)

bass_utils.run_command = _patched_run_command

f32 = mybir.dt.float32
f32r = mybir.dt.float32r
AF = mybir.ActivationFunctionType
ALU = mybir.AluOpType

B, S, D, H, DFF = 4, 1024, 1024, 16, 4096
DEPTH = D // H
R = 512          # rows (query block) per core
EPS = 1e-6
N_CORES = 8


def _split_waits(nc, maxw=1):
    """Walrus in this toolchain encodes at most one semaphore wait per
    instruction; Tile emits several. Move excess waits onto same-engine NOPs
    placed immediately before the instruction (sequential per-engine streams
    make this equivalent)."""
    for f in nc.m.functions:
        for bb in f.blocks:
            out = []
            for inst in bb.instructions:
                si = inst.sync_info
                if si is not None and len(si.on_wait) > maxw:
                    waits = list(si.on_wait)
                    keep, excess = waits[-maxw:], waits[:-maxw]
                    eng = getattr(inst, "engine", None)
                    k = 0
                    while excess:
                        chunk, excess = excess[:maxw], excess[maxw:]
                        out.append(mybir.InstNoOp(
                            name=f"{inst.name}_wsp{k}",
                            engine=eng,
                            bass_nofuse=True,
                            sync_info=mybir.SyncInfo(on_wait=chunk, on_update=[]),
                        ))
                        k += 1
                    inst.sync_info = mybir.SyncInfo(
                        on_wait=keep, on_update=list(si.on_update))
                out.append(inst)
            bb.instructions = out


def build_program():
    nc = bass.Bass("TRN2", target_bir_lowering=False, debug=False)

    def din(name, shape):
        return nc.dram_tensor(name, shape, f32, kind="ExternalInput").ap()

    dc_own_d = din("dc_own", [D, R])        # dec_input own rows, transposed
    dke_d = din("dke", [D, S])              # dec keys (reordered: ctx|diag), transposed
    enc_d = din("encT", [D, S])             # enc_output, transposed
    mask_d = din("maskT", [R, R])           # causal diag block, [key, q], pre * -8e9
    padb_d = din("padb", [128, 8])          # -1e9 * padding_mask, chunked
    vm1_d = din("vm1", [128, 8])            # self V-row mask (chunked)
    vm1r_d = din("vm1r", [128, 8, 16])      # same, replicated per head
    vm2_d = din("vm2", [128, 8])            # ones
    vm2r_d = din("vm2r", [128, 8, 16])      # ones
    ones_d = din("onesd", [128, 128])       # ones
    w_d = {k: din(k, [D, D]) for k in ("wq1", "wk1", "wq2", "wk2", "wo1", "wo2")}
    fw1_d = din("fw1", [D, DFF])
    fw2_d = din("fw2", [DFF, D])
    bc_d = {k: din(k, [128, 8]) for k in
            ("bq1c", "bk1c", "bq2c", "bk2c", "bo1c", "bo2c", "fb2c",
             "g1c", "b1c", "g2c", "b2c", "g3c", "b3c")}
    fb1c_d = din("fb1c", [128, 32])
    out_d = nc.dram_tensor("outT", [D, R], f32, kind="ExternalOutput").ap()

    with tile.TileContext(nc) as tc:
        with tc.tile_pool(name="persist", bufs=1) as pp, \
             tc.tile_pool(name="consts", bufs=1) as cp:
            # ---- persistent SBUF ----
            arena = pp.tile([128, 16384], f32r, name="arena")     # 8 MiB
            dke = arena[:, 0:8192].rearrange("p (f r) -> p f r", f=8)
            kTv = arena[:, 8192:16384].rearrange("p (f r) -> p f r", f=8)
            vaug = pp.tile([128, 8, 16, 65], f32r, name="vaug")
            dc_own = pp.tile([128, 8, R], f32r, name="dc_own")
            qbuf = pp.tile([128, 8, R], f32r, name="qbuf")        # q1T -> q2T
            abufA = pp.tile([128, 8, R], f32r, name="abufA")      # attn1T -> x2pre/x2T
            xa = pp.tile([128, 8, R], f32r, name="xa")            # x1pre/x1T -> x3pre
            # phase-3/4 views of the arena: enc goes where k1T lived, k2T where
            # dec-keys lived, attn2T into the (then-dead) enc region.
            kT2v = arena[:, 0:8192].rearrange("p (f r) -> p f r", f=8)
            encv = arena[:, 8192:16384].rearrange("p (f r) -> p f r", f=8)
            abufB = arena[:, 8192:12288].rearrange("p (f r) -> p f r", f=8)

            # ---- constants ----
            onesb = cp.tile([128, 128], f32r, name="onesb")
            padb = cp.tile([128, 8], f32, name="padb")
            vm1 = cp.tile([128, 8], f32, name="vm1")
            vm2 = cp.tile([128, 8], f32, name="vm2")
            bcs = {k: cp.tile([128, 8], f32, name=k) for k in bc_d}
            fb1c = cp.tile([128, 32], f32, name="fb1c")

            nc.sync.dma_start(out=onesb, in_=ones_d.bitcast(f32r))
            nc.sync.dma_start(out=padb, in_=padb_d)
            nc.sync.dma_start(out=vm1, in_=vm1_d)
            nc.sync.dma_start(out=vm2, in_=vm2_d)
            for k in bcs:
                nc.sync.dma_start(out=bcs[k], in_=bc_d[k])
            nc.sync.dma_start(out=fb1c, in_=fb1c_d)
            nc.sync.dma_start(out=dc_own,
                              in_=dc_own_d.rearrange("(f p) r -> p f r", p=128).bitcast(f32r))
            nc.sync.dma_start(out=dke,
                              in_=dke_d.rearrange("(f p) r -> p f r", p=128).bitcast(f32r))

            ones1 = onesb[0:1, :]      # [1, 128] f32r
            onesp = onesb[:, 0:1]      # [128, 1] f32r

            # ---- helpers ----
            def gemm_TN(Wd, xt, KCn, MCn, NN, drain, wp, ps):
                """OUT^T[m-chunk, n] = sum_kc W[kc, m]^T @ xt(kc, n).
                xt(kc, n) -> [128, 512] f32r AP. drain(mi, n, psum_ap)."""
                g = max(1, 4 // NN)
                for mg in range(0, MCn, g):
                    gs = min(g, MCn - mg)
                    pps = {}
                    for i in range(gs):
                        for n in range(NN):
                            pps[(i, n)] = ps.tile([128, 512], f32, name="pp")
                    for kc in range(KCn):
                        wt = wp.tile([128, gs * 128], f32r, name="wt")
                        nc.sync.dma_start(
                            out=wt,
                            in_=Wd[kc * 128:(kc + 1) * 128,
                                   mg * 128:(mg + gs) * 128].bitcast(f32r))
                        for i in range(gs):
                            for n in range(NN):
                                nc.tensor.matmul(
                                    pps[(i, n)][:],
                                    lhsT=wt[:, i * 128:(i + 1) * 128],
                                    rhs=xt(kc, n),
                                    start=(kc == 0), stop=(kc == KCn - 1))
                    for i in range(gs):
                        for n in range(NN):
                            drain(mg + i, n, pps[(i, n)])

            def gemm_NT(Wd, xt_sb, KCn, RCn, NFn, drain, wp, ps):
                """OUT[r-chunk] = X @ W : lhsT = xt chunks, rhs = W cols.
                drain(rc, nf, psum_ap). xt_sb [128, KCn, S] f32r."""
                for nf in range(NFn):
                    for rg in range(0, RCn, 4):
                        gs = min(4, RCn - rg)
                        pps = [ps.tile([128, 512], f32, name="pp") for _ in range(gs)]
                        for kc in range(KCn):
                            wt = wp.tile([128, 512], f32r, name="wt")
                            nc.sync.dma_start(
                                out=wt,
                                in_=Wd[kc * 128:(kc + 1) * 128,
                                       nf * 512:(nf + 1) * 512].bitcast(f32r))
                            for i in range(gs):
                                nc.tensor.matmul(
                                    pps[i][:],
                                    lhsT=xt_sb[:, kc, (rg + i) * 128:(rg + i + 1) * 128],
                                    rhs=wt[:],
                                    start=(kc == 0), stop=(kc == KCn - 1))
                        for i in range(gs):
                            drain(rg + i, nf, pps[i])

            def attention(q_sb, kT_sb, v_sb, attn_out, is_self, masks, spool,
                          epool, upool, ps_s, ps_av, ps_b):
                for f in range(8):
                    avs = [ps_av.tile([65, 512], f32, name="av") for _ in range(2)]
                    for kc in range(8):
                        ss = ps_s.tile([128, 1024], f32, name="ss")
                        for a in range(2):
                            nc.tensor.matmul(
                                ss[:, a * 512:(a + 1) * 512],
                                lhsT=kT_sb[64 * a:64 * (a + 1), f, kc * 128:(kc + 1) * 128],
                                rhs=q_sb[64 * a:64 * (a + 1), f, :],
                                start=True, stop=True)
                        if is_self and kc >= 4:
                            nc.vector.tensor_tensor(
                                out=ss[:].rearrange("p (a r) -> p a r", a=2),
                                in0=ss[:].rearrange("p (a r) -> p a r", a=2),
                                in1=bass.AP(tensor=masks.tensor,
                                            offset=masks[:, kc - 4, :].offset,
                                            ap=[list(masks.ap[0]), [0, 2],
                                                list(masks.ap[2])]),
                                op=ALU.add)
                        e = epool.tile([128, 1024], f32r, name="ee")
                        bias = 0.0 if is_self else padb[:, kc:kc + 1]
                        nc.scalar.activation(e[:], ss[:], AF.Exp,
                                             bias=bias, scale=0.125)
                        for a in range(2):
                            nc.tensor.matmul(
                                avs[a][:],
                                lhsT=v_sb[:, kc, 2 * f + a, :],
                                rhs=e[:, a * 512:(a + 1) * 512],
                                start=(kc == 0), stop=(kc == 7))
                    for a in range(2):
                        rec = upool.tile([1, 512], f32r, name="rec")
                        with nc.allow_low_precision(reason="f32r keeps fp32 bits"):
                            nc.vector.reciprocal(rec[:], avs[a][64:65, :])
                        dst = attn_out[64 * a:64 * (a + 1), f, :]
                        nc.scalar.copy(dst, avs[a][0:64, :])
                        bp = ps_b.tile([64, 512], f32, name="bp")
                        nc.tensor.matmul(bp[:], lhsT=onesb[0:1, 0:64], rhs=rec[:],
                                         start=True, stop=True)
                        with nc.allow_low_precision(reason="f32r keeps fp32 bits"):
                            nc.vector.tensor_mul(dst, dst, bp[:])

            def layernorm(x_sb, gC, bC, out_sb, sqp, ltp, ps_ln):
                pm = ps_ln.tile([1, 512], f32, name="pm")
                pv = ps_ln.tile([1, 512], f32, name="pv")
                for kc in range(8):
                    nc.tensor.matmul(pm[:], lhsT=onesp, rhs=x_sb[:, kc, :],
                                     start=(kc == 0), stop=(kc == 7))
                    sq = sqp.tile([128, 512], f32r, name="sq")
                    nc.scalar.activation(sq[:], x_sb[:, kc, :], AF.Square)
                    nc.tensor.matmul(pv[:], lhsT=onesp, rhs=sq[:],
                                     start=(kc == 0), stop=(kc == 7))
                m = ltp.tile([1, 512], f32, name="lm")
                sc = ltp.tile([1, 512], f32, name="lsc")
                sc2 = ltp.tile([1, 512], f32, name="lsc2")
                inv = ltp.tile([1, 512], f32r, name="linv")
                minv = ltp.tile([1, 512], f32r, name="lminv")
                nc.vector.tensor_scalar_mul(m[:], pm[:], 1.0 / D)
                nc.vector.tensor_scalar_mul(sc[:], pv[:], 1.0 / D)   # E[x^2]
                nc.vector.tensor_mul(sc2[:], m[:], m[:])             # m^2
                nc.vector.tensor_scalar_add(sc2[:], sc2[:], -EPS)
                nc.vector.tensor_tensor(out=sc[:], in0=sc[:], in1=sc2[:],
                                        op=ALU.subtract)             # var + eps
                nc.scalar.activation(sc[:], sc[:], AF.Sqrt)
                with nc.allow_low_precision(reason="f32r keeps fp32 bits"):
                    nc.vector.reciprocal(inv[:], sc[:])
                    nc.vector.tensor_mul(minv[:], m[:], inv[:])
                binv = ps_ln.tile([128, 512], f32, name="binv")
                bmv = ps_ln.tile([128, 512], f32, name="bmv")
                nc.tensor.matmul(binv[:], lhsT=ones1, rhs=inv[:], start=True, stop=True)
                nc.tensor.matmul(bmv[:], lhsT=ones1, rhs=minv[:], start=True, stop=True)
                def bc8(ps_t):
                    return bass.AP(tensor=ps_t.tensor, offset=ps_t.offset,
                                   ap=[list(ps_t.ap[0]), [0, 8], list(ps_t.ap[1])])
                with nc.allow_low_precision(reason="f32r keeps fp32 bits"):
                    nc.vector.tensor_tensor(out=x_sb[:], in0=x_sb[:], in1=bc8(binv),
                                            op=ALU.mult)
                    nc.vector.tensor_tensor(out=x_sb[:], in0=x_sb[:], in1=bc8(bmv),
                                            op=ALU.subtract)
                for kc in range(8):
                    nc.scalar.activation(out_sb[:, kc, :], x_sb[:, kc, :], AF.Identity,
                                         bias=bC[:, kc:kc + 1], scale=gC[:, kc:kc + 1])

            # ================= phase 1: self projections =================
            with tc.tile_pool(name="w1", bufs=6) as wp, \
                 tc.tile_pool(name="ps1", bufs=6, space="PSUM") as ps:

                def drain_q1(mi, n, pa):
                    nc.scalar.activation(qbuf[:, mi, :], pa[:], AF.Identity,
                                         bias=bcs["bq1c"][:, mi:mi + 1])
                gemm_TN(w_d["wq1"], lambda kc, n: dc_own[:, kc, :], 8, 8, 1,
                        drain_q1, wp, ps)

                def drain_k1(mi, n, pa):
                    nc.scalar.activation(kTv[:, mi, n * 512:(n + 1) * 512], pa[:],
                                         AF.Identity, bias=bcs["bk1c"][:, mi:mi + 1])
                gemm_TN(w_d["wk1"], lambda kc, n: dke[:, kc, n * 512:(n + 1) * 512],
                        8, 8, 2, drain_k1, wp, ps)

                def drain_v1(rc, nf, pa):
                    dst = vaug[:, rc, nf * 8:(nf + 1) * 8, 0:64]
                    src = pa[:].rearrange("p (h d) -> p h d", h=8)
                    nc.scalar.activation(dst, src, AF.Copy, scale=vm1[:, rc:rc + 1])
                gemm_NT(w_d["wq1"], dke, 8, 8, 2, drain_v1, wp, ps)
                for rc in range(8):
                    nc.sync.dma_start(out=vaug[:, rc, :, 64:65],
                                      in_=vm1r_d[:, rc, :].bitcast(f32r))

            # ================= phase 2: self attention =================
            with tc.tile_pool(name="mk2", bufs=1) as mkp, \
                 tc.tile_pool(name="sp2", bufs=2) as spool, \
                 tc.tile_pool(name="ep2", bufs=3) as epool, \
                 tc.tile_pool(name="up2", bufs=2) as upool, \
                 tc.tile_pool(name="pss", bufs=2, space="PSUM") as ps_s, \
                 tc.tile_pool(name="psav", bufs=2, space="PSUM") as ps_av, \
                 tc.tile_pool(name="psb", bufs=2, space="PSUM") as ps_b:
                masks = mkp.tile([128, 4, R], f32, name="masks")
                for c in range(4):
                    nc.sync.dma_start(out=masks[:, c, :],
                                      in_=mask_d[c * 128:(c + 1) * 128, :])
                attention(qbuf, kTv, vaug, abufA, True, masks, spool, epool,
                          upool, ps_s, ps_av, ps_b)

            # ================= phase 3: cross projections =================
            with tc.tile_pool(name="w3", bufs=6) as wp, \
                 tc.tile_pool(name="ps3", bufs=6, space="PSUM") as ps:
                nc.sync.dma_start(out=encv,
                                  in_=enc_d.rearrange("(f p) r -> p f r", p=128).bitcast(f32r))

                def drain_q2(mi, n, pa):
                    nc.scalar.activation(qbuf[:, mi, :], pa[:], AF.Identity,
                                         bias=bcs["bq2c"][:, mi:mi + 1])
                gemm_TN(w_d["wq2"], lambda kc, n: dc_own[:, kc, :], 8, 8, 1,
                        drain_q2, wp, ps)

                def drain_k2(mi, n, pa):
                    nc.scalar.activation(kT2v[:, mi, n * 512:(n + 1) * 512], pa[:],
                                         AF.Identity, bias=bcs["bk2c"][:, mi:mi + 1])
                gemm_TN(w_d["wk2"], lambda kc, n: encv[:, kc, n * 512:(n + 1) * 512],
                        8, 8, 2, drain_k2, wp, ps)

                def drain_v2(rc, nf, pa):
                    dst = vaug[:, rc, nf * 8:(nf + 1) * 8, 0:64]
                    src = pa[:].rearrange("p (h d) -> p h d", h=8)
                    nc.scalar.activation(dst, src, AF.Copy, scale=vm2[:, rc:rc + 1])
                gemm_NT(w_d["wq2"], encv, 8, 8, 2, drain_v2, wp, ps)
                for rc in range(8):
                    nc.sync.dma_start(out=vaug[:, rc, :, 64:65],
                                      in_=vm2r_d[:, rc, :].bitcast(f32r))

            # ================= phase 4: cross attention =================
            with tc.tile_pool(name="sp4", bufs=2) as spool, \
                 tc.tile_pool(name="ep4", bufs=3) as epool, \
                 tc.tile_pool(name="up4", bufs=2) as upool, \
                 tc.tile_pool(name="pss4", bufs=2, space="PSUM") as ps_s, \
                 tc.tile_pool(name="psav4", bufs=2, space="PSUM") as ps_av, \
                 tc.tile_pool(name="psb4", bufs=2, space="PSUM") as ps_b:
                attention(qbuf, kT2v, vaug, abufB, False, None, spool, epool,
                          upool, ps_s, ps_av, ps_b)

            # ============ phase 5: output projections + LN1/LN2 ============
            with tc.tile_pool(name="w5", bufs=6) as wp, \
                 tc.tile_pool(name="tw5", bufs=2) as twp, \
                 tc.tile_pool(name="sq5", bufs=2) as sqp, \
                 tc.tile_pool(name="lt5", bufs=1) as ltp, \
                 tc.tile_pool(name="ps5", bufs=4, space="PSUM") as ps, \
                 tc.tile_pool(name="ps5ln", bufs=1, space="PSUM") as ps_ln:

                def drain_wo1(mi, n, pa):
                    tw = twp.tile([128, 512], f32, name="tw")
                    nc.scalar.activation(tw[:], pa[:], AF.Identity,
                                         bias=bcs["bo1c"][:, mi:mi + 1])
                    with nc.allow_low_precision(reason="f32r keeps fp32 bits"):
                        nc.vector.tensor_add(xa[:, mi, :], tw[:], dc_own[:, mi, :])
                gemm_TN(w_d["wo1"], lambda kc, n: abufA[:, kc, :], 8, 8, 1,
                        drain_wo1, wp, ps)

                layernorm(xa, bcs["g1c"], bcs["b1c"], xa, sqp, ltp, ps_ln)

                def drain_wo2(mi, n, pa):
                    tw = twp.tile([128, 512], f32, name="tw")
                    nc.scalar.activation(tw[:], pa[:], AF.Identity,
                                         bias=bcs["bo2c"][:, mi:mi + 1])
                    with nc.allow_low_precision(reason="f32r keeps fp32 bits"):
                        nc.vector.tensor_add(abufA[:, mi, :], tw[:], xa[:, mi, :])
                gemm_TN(w_d["wo2"], lambda kc, n: abufB[:, kc, :], 8, 8, 1,
                        drain_wo2, wp, ps)

                layernorm(abufA, bcs["g2c"], bcs["b2c"], abufA, sqp, ltp, ps_ln)

            # ================= phase 6: FFN + LN3 + output =================
            with tc.tile_pool(name="w6", bufs=6) as wp, \
                 tc.tile_pool(name="tw6", bufs=2) as twp, \
                 tc.tile_pool(name="ps6", bufs=4, space="PSUM") as ps, \
                 tc.tile_pool(name="ps6b", bufs=1, space="PSUM") as ps8:

                def drain_f1(mi, n, pa):
                    nc.scalar.activation(arena[:, mi * 512:(mi + 1) * 512], pa[:],
                                         AF.Relu, bias=fb1c[:, mi:mi + 1])
                gemm_TN(fw1_d, lambda kc, n: abufA[:, kc, :], 8, 32, 1,
                        drain_f1, wp, ps)

                # ffn2 in two 4-bank halves so it shares PSUM with ffn1 and
                # its matmuls can fill ffn1's weight-DMA gaps.
                for mh in range(2):
                    pps = [ps8.tile([128, 512], f32, name=f"pf_{i}")
                           for i in range(4)]
                    for kc in range(32):
                        wt = wp.tile([128, 512], f32r, name="wt")
                        nc.sync.dma_start(
                            out=wt,
                            in_=fw2_d[kc * 128:(kc + 1) * 128,
                                      mh * 512:(mh + 1) * 512].bitcast(f32r))
                        for i in range(4):
                            nc.tensor.matmul(
                                pps[i][:],
                                lhsT=wt[:, i * 128:(i + 1) * 128],
                                rhs=arena[:, kc * 512:(kc + 1) * 512],
                                start=(kc == 0), stop=(kc == 31))
                    for i in range(4):
                        mi = mh * 4 + i
                        tw = twp.tile([128, 512], f32, name="tw")
                        nc.scalar.activation(tw[:], pps[i][:], AF.Identity,
                                             bias=bcs["fb2c"][:, mi:mi + 1])
                        with nc.allow_low_precision(reason="f32r keeps fp32 bits"):
                            nc.vector.tensor_add(xa[:, mi, :], tw[:], abufA[:, mi, :])

            with tc.tile_pool(name="sq7", bufs=2) as sqp, \
                 tc.tile_pool(name="lt7", bufs=1) as ltp, \
                 tc.tile_pool(name="ps7ln", bufs=1, space="PSUM") as ps_ln:
                layernorm(xa, bcs["g3c"], bcs["b3c"], qbuf, sqp, ltp, ps_ln)
                for mi in range(8):
                    nc.sync.dma_start(out=out_d[mi * 128:(mi + 1) * 128, :].bitcast(f32r),
                                      in_=qbuf[:, mi, :])

    _split_waits(nc, 1)
    return nc


_PROGRAM = None


def _get_program():
    global _PROGRAM
    if _PROGRAM is None:
        _PROGRAM = build_program()
    return _PROGRAM


def _core_inputs(inp, c):
    b, j = c // 2, c % 2
    dec = np.asarray(inp["dec_input"][b], np.float32)      # [S, D]
    enc = np.asarray(inp["enc_output"][b], np.float32)
    decT = np.ascontiguousarray(dec.T)                     # [D, S]
    own = np.ascontiguousarray(decT[:, j * R:(j + 1) * R])
    if j == 1:
        dke = decT                                         # ctx = rows 0:512, diag = 512:1024
    else:
        dke = np.ascontiguousarray(
            np.concatenate([decT[:, R:], decT[:, :R]], axis=1))
    la = np.asarray(inp["look_ahead_mask"], np.float32)[0, 0]
    maskT = np.ascontiguousarray(la[j * R:(j + 1) * R, j * R:(j + 1) * R].T) * np.float32(-8e9)
    padb = (np.asarray(inp["padding_mask"], np.float32)[b, 0, 0] * np.float32(-1e9))
    vm = np.ones(S, np.float32)
    if j == 0:
        vm[:R] = 0.0                                       # ctx block invalid for first half
    v2 = np.ones(S, np.float32)

    def chunk(a, n):
        return np.ascontiguousarray(np.asarray(a, np.float32).reshape(n, 128).T)

    wo1 = np.asarray(inp["wo1"], np.float32)
    wo2 = np.asarray(inp["wo2"], np.float32)
    bo1e = np.asarray(inp["bq1"], np.float32) @ wo1 + np.asarray(inp["bo1"], np.float32)
    bo2e = np.asarray(inp["bq2"], np.float32) @ wo2 + np.asarray(inp["bo2"], np.float32)

    return {
        "dc_own": own, "dke": dke,
        "encT": np.ascontiguousarray(enc.T),
        "maskT": maskT,
        "padb": chunk(padb, 8),
        "vm1": chunk(vm, 8),
        "vm1r": np.repeat(chunk(vm, 8)[:, :, None], 16, axis=2),
        "vm2": chunk(v2, 8),
        "vm2r": np.ones((128, 8, 16), np.float32),
        "onesd": np.ones((128, 128), np.float32),
        "wq1": np.asarray(inp["wq1"], np.float32),
        "wk1": np.asarray(inp["wk1"], np.float32),
        "wq2": np.asarray(inp["wq2"], np.float32),
        "wk2": np.asarray(inp["wk2"], np.float32),
        "wo1": wo1, "wo2": wo2,
        "fw1": np.asarray(inp["ff_w1"], np.float32),
        "fw2": np.asarray(inp["ff_w2"], np.float32),
        "bq1c": chunk(inp["bq1"], 8), "bk1c": chunk(inp["bk1"], 8),
        "bq2c": chunk(inp["bq2"], 8), "bk2c": chunk(inp["bk2"], 8),
        "bo1c": chunk(bo1e, 8), "bo2c": chunk(bo2e, 8),
        "fb1c": chunk(inp["ff_b1"], 32), "fb2c": chunk(inp["ff_b2"], 8),
        "g1c": chunk(inp["ln1_g"], 8), "b1c": chunk(inp["ln1_b"], 8),
        "g2c": chunk(inp["ln2_g"], 8), "b2c": chunk(inp["ln2_b"], 8),
        "g3c": chunk(inp["ln3_g"], 8), "b3c": chunk(inp["ln3_b"], 8),
    }


def kernel(**inputs):
    nc = _get_program()
    in_maps = [_core_inputs(inputs, c) for c in range(N_CORES)]
    res = run_bass_kernel_spmd(nc, in_maps, list(range(N_CORES)))
    out = np.empty((B, S, D), np.float32)
    for c in range(N_CORES):
        b, j = c // 2, c % 2
        out[b, j * R:(j + 1) * R, :] = res.results[c]["outT"].T
    return out


if __name__ == "__main__":
    import tempfile
    from concourse.bass_utils import compile_bass_kernel
    nc = build_program()
    with tempfile.TemporaryDirectory() as td:
        compile_bass_kernel(nc, td)
    print("COMPILE OK")
